# revision 36
# baseline (speedup 1.0000x reference)
"""Distributed causal multi-head attention for 8 Trainium2 NeuronCores.

Problem: B=2, S=2048, D=1024, H=16 heads (hd=64), fp32.
    qkv = x @ w_qkv + b_qkv ; causal softmax attention ; out = attn @ w_proj + b_proj

Distribution: core c -> (batch b = c//4, head group g = c%4 -> heads [4g, 4g+4)).
Transposed dataflow (channels on partitions, sequence on free axis); x arrives
host-transposed.

Restructured from the v1 phase-serial kernel (~245-257us) to ~215-225us.
Key findings from trace analysis, baked into this version:
  - k-phase output slots: each k-phase's attention output ships in 64-row
    "slots": receiver core c gets, per phase k, rows
    [512*(c//2) + 128*k + 64*(c%2), +64) of both batches.  A2A#1 carries
    phases {0,1} (doorbell ~mid-attention), A2A#2 carries {2,3} (doorbell at
    attention end).  Collectives cost ~15-25us of data movement PLUS ~27us
    of queue-release lag on the serializing GpSimd queue, and consumers see
    completion ~5us after the data phase ends; two A2As with all projection
    work at the end is the best overlap under those constants (3-4 smaller
    A2As cascade on the queue lag; 4-core-group A2As are unsupported - mesh
    needs >4 ranks).
  - A tiny warm-up AllGather issues at t~0: the FIRST collective pays
    ~55-60us extra firmware warm-up; this absorbs it off the critical path.
    Nothing else may use the GpSimd queue (it blocks behind collectives),
    so the V' ones-memset runs on Vector.
  - Projection packs batch0 rows on partitions 0:64 and batch1 on 64:128 so
    proj matmuls keep full 128-partition output despite 64-row slots.
  - The causal mask is applied INSIDE the score PSUM accumulation as an
    extra matmul (mask^T stationary x identity moving) on diagonal tiles:
    a Vector-engine mask add would queue behind the normalize chains and
    delay exp by up to ~3us per group (the Vector queue is in-order).
  - Each group's last 3 PV chunks and its normalize chain are carried into
    the NEXT group as its first 4 fillers, so neither the PE nor Scalar
    idles at group boundaries (boundary stalls also reset the PE p-state:
    it runs at 1.2GHz until ~3us of continuous execution, 2.4GHz after).
  - The softmax 1/sum broadcast matmul is bf16 (fp32 matmul is 4 cyc/row:
    853ns x16 = 13.6us of PE busy; bf16 is 213ns).
  - qk pair-1 and V-tile projections run as fillers inside the first two
    groups (pair-swapped order (0,0),(0,1),(1,0),(1,1),...), shrinking the
    serial pre-attention head.
  - pjT fetches for A2A#2 slots issue on the Scalar queue so they cannot
    head-block staging/out DMAs on the sync queue; pjT fetches for A2A#1
    prefetch on sync right after the collective.
  - Priority-ordered DMA with a small first chunk (wqk d0 + x slab0 d0);
    output ships as bf16 (host casts back to f32).
"""

import os
import sys
from collections import deque

sys.path.insert(0, "/opt/trn_rl_repo")

import numpy as np

import concourse.bass as bass
import concourse.tile as tile
from concourse import bacc, mybir
from concourse.bass_utils import run_bass_kernel_spmd

B, S, D = 2, 2048, 1024
H = 16
HD = 64
P = 128
N_CORES = 8
HPC = 4           # heads per core
DCH = D // P      # 8 contraction chunks
NQT = S // P      # 16 q tiles of 128
SCALE = 1.0 / 8.0  # 1/sqrt(hd)
NEG = -1.0e9

F32 = mybir.dt.float32
BF16 = mybir.dt.bfloat16


def attn_chunks(k):
    """Pack the kv-tile iterations of q-group k into <=512-col score chunks.

    First-fit: a late short tile (N=128) fills the slack of an earlier
    384-col chunk, so each chunk (= one exp call) is as full as possible."""
    T = 13 + k
    chunks = []  # [entries, used]
    for t in range(T):
        s0 = max(0, (t - k + 3) // 4)
        N = (4 - s0) * P
        for ch in chunks:
            if ch[1] + N <= 512:
                ch[0].append((t, ch[1], N, s0))
                ch[1] += N
                break
        else:
            chunks.append([[(t, 0, N, s0)], N])
    out = [c[0] for c in chunks]
    # split the final 512-col chunk into two halves: its exp call then
    # finishes sooner after its scores, so the next group's second score
    # chunk never stalls on the 2-deep PSUM score ring
    last = out[-1]
    if len(last) == 2:
        out[-1] = [last[0]]
        out.append([(last[1][0], 0, last[1][2], last[1][3])])
    return out


def build():
    nc = bacc.Bacc(num_devices=N_CORES)

    xT = nc.declare_dram_parameter("xT", [D, S], BF16, isOutput=False)
    # columns reordered host-side: [q_p0 | k_p0 | q_p1 | k_p1] (128 each)
    w_qk = nc.declare_dram_parameter("w_qk", [D, 2 * HPC * HD], BF16, isOutput=False)
    w_v = nc.declare_dram_parameter("w_v", [D, HPC * HD], BF16, isOutput=False)
    consts = nc.declare_dram_parameter("consts", [P, 4], F32, isOutput=False)
    # mi[:, 0:128] = mask^T (0 where q>=kv else NEG), mi[:, 128:256] = identity
    mi = nc.declare_dram_parameter("mi", [P, 2 * P], BF16, isOutput=False)
    b_v = nc.declare_dram_parameter("b_v", [1, HPC * HD], BF16, isOutput=False)
    w_proj = nc.declare_dram_parameter("w_proj", [D, D], BF16, isOutput=False)
    b_proj = nc.declare_dram_parameter("b_proj", [1, D], BF16, isOutput=False)
    # slot k: rows [512*(c//2) + 128*k + 64*(c%2), +64); partitions 0:64 = b0,
    # 64:128 = b1
    out_ext = nc.declare_dram_parameter("out", [4, P, D], BF16, isOutput=True)

    groups = [list(range(N_CORES))]

    with tile.TileContext(nc) as tc:
        with (
            tc.tile_pool(name="weights", bufs=1) as wpool,
            tc.tile_pool(name="xslab", bufs=4) as xpool,
            tc.tile_pool(name="qkT", bufs=1) as qkpool,
            tc.tile_pool(name="big", bufs=1) as bigpool,
            tc.tile_pool(name="prob", bufs=7) as ppool,
            tc.tile_pool(name="small", bufs=4) as spool,
            tc.tile_pool(name="pj", bufs=4) as pjpool,
            tc.tile_pool(name="dram", bufs=1, space="DRAM") as dpool,
            tc.tile_pool(name="psS", bufs=2, space="PSUM") as psS,   # scores 2 banks/slot
            tc.tile_pool(name="psV", bufs=2, space="PSUM") as psV,   # pv accumulators
            tc.tile_pool(name="psQ", bufs=2, space="PSUM") as psQ,   # qkv/proj groups
        ):
            # a2a layout: [dest core, bslot*256 + pair*128 + hd_part, 64 rows]
            # A2A#1 carries phases {0,1} (doorbell ~mid-attention; its ~27us
            # queue-release lag mostly clears before A2A#2's data exists);
            # A2A#2 carries phases {2,3} at attention end.
            a2a_in1 = dpool.tile([N_CORES, 512, 64], BF16, tag="a2a_in1")
            a2a_out1 = dpool.tile([N_CORES, 512, 64], BF16, tag="a2a_out1")
            a2a_in2 = dpool.tile([N_CORES, 512, 64], BF16, tag="a2a_in2")
            a2a_out2 = dpool.tile([N_CORES, 512, 64], BF16, tag="a2a_out2")
            dummy_in = dpool.tile([1, 16], BF16, tag="dummy_in")
            dummy_out = dpool.tile([N_CORES, 16], BF16, tag="dummy_out")

            # tiny warm-up collective issued immediately: the collectives
            # firmware has a large first-collective warm-up cost (observed:
            # the first real A2A would not move data before ~145us regardless
            # of when its inputs were staged); this absorbs it off the
            # critical path.
            nc.sync.dma_start(out=dummy_in[:], in_=b_v[:, 0:16])
            nc.gpsimd.collective_compute(
                "AllGather", mybir.AluOpType.bypass,
                ins=[dummy_in[:].opt()], outs=[dummy_out[:].opt()],
                replica_groups=groups,
            )

            # ---- DMA priority order ----
            wqk_sb = wpool.tile([P, DCH, 2 * HPC * HD], BF16)
            xsl_list = [
                xpool.tile([P, DCH, 512], BF16, tag="xslab", name=f"xsl{st}")
                for st in range(4)
            ]
            # smallest possible first chunks so the first matmul starts early
            nc.sync.dma_start(
                out=wqk_sb[:, 0:1, 0:256],
                in_=w_qk[:, 0:256].rearrange("(o p) c -> p o c", p=P)[:, 0:1, :],
            )
            nc.sync.dma_start(
                out=xsl_list[0][:, 0:1, :],
                in_=xT[:, :].rearrange("(o p) s -> p o s", p=P)[:, 0:1, 0:512],
            )
            consts_sb = wpool.tile([P, 4], F32)
            nc.sync.dma_start(out=consts_sb[:], in_=consts[:, :])
            nc.sync.dma_start(
                out=wqk_sb[:, 1:4, 0:256],
                in_=w_qk[:, 0:256].rearrange("(o p) c -> p o c", p=P)[:, 1:4, :],
            )
            nc.sync.dma_start(
                out=xsl_list[0][:, 1:4, :],
                in_=xT[:, :].rearrange("(o p) s -> p o s", p=P)[:, 1:4, 0:512],
            )
            nc.sync.dma_start(
                out=wqk_sb[:, 4:8, 0:256],
                in_=w_qk[:, 0:256].rearrange("(o p) c -> p o c", p=P)[:, 4:8, :],
            )
            nc.sync.dma_start(
                out=xsl_list[0][:, 4:8, :],
                in_=xT[:, :].rearrange("(o p) s -> p o s", p=P)[:, 4:8, 0:512],
            )
            mi_sb = wpool.tile([P, 2 * P], BF16)
            nc.sync.dma_start(out=mi_sb[:], in_=mi[:, :])
            nc.sync.dma_start(
                out=wqk_sb[:, :, 256:512],
                in_=w_qk[:, 256:512].rearrange("(o p) c -> p o c", p=P),
            )
            for st in range(1, 4):
                for dh in range(2):
                    dsl = slice(dh * 4, dh * 4 + 4)
                    nc.sync.dma_start(
                        out=xsl_list[st][:, dsl, :],
                        in_=xT[:, :].rearrange("(o p) s -> p o s", p=P)[:, dsl, st * 512:(st + 1) * 512],
                    )
            wv_sb = wpool.tile([P, DCH, HPC * HD], BF16)
            nc.sync.dma_start(out=wv_sb[:], in_=w_v[:, :].rearrange("(o p) c -> p o c", p=P))
            bv_sb = wpool.tile([1, HPC * HD], BF16)
            nc.sync.dma_start(out=bv_sb[:], in_=b_v[:, :])
            wproj_sb = wpool.tile([P, DCH, D], BF16)
            nc.sync.dma_start(out=wproj_sb[:], in_=w_proj[:, :].rearrange("(o p) c -> p o c", p=P))
            bproj_sb = wpool.tile([1, D], BF16)
            nc.sync.dma_start(out=bproj_sb[:], in_=b_proj[:, :])

            bqk_sb = consts_sb[:, 0:4]
            maskT_sb = mi_sb[:, 0:P]
            ident_sb = mi_sb[:, P:2 * P]
            ones_sb = wpool.tile([1, P], BF16)
            nc.vector.memset(ones_sb[:], 1.0)
            warm_sb = wpool.tile([1, 512], BF16)
            nc.vector.memset(warm_sb[:], 1.0)
            # ~10 throwaway matmuls: the PE clock needs ~3us of continuous
            # execution to ramp 0.65->2.4GHz; these run while the first
            # weight DMAs land so the real qk units start at speed
            for _ in range(10):
                wps = psQ.tile([P, 512], F32, tag="mm", name="warm")
                nc.tensor.matmul(
                    wps[:], ones_sb[:, :], warm_sb[:, :], start=True, stop=True,
                )

            # qkT layout: [128, ct, 2048]; ct: 0=q_p0, 1=k_p0, 2=q_p1, 3=k_p1
            qkT_sb = qkpool.tile([P, 4, S], BF16)
            # V': [128 kv_inner, 16 kv_outer, 4*65]; col 65h+64 = 1.0 (softmax denom)
            # memset on Vector, NOT GpSimd: the GpSimd queue must stay free
            # for collectives (the warm-up AllGather may occupy it for a
            # long time)
            v1_sb = bigpool.tile([P, NQT, HPC * 65], BF16)
            nc.vector.memset(v1_sb[:], 1.0)
            # attn outT: [128 (2 heads x 64), pair, 2048] bf16; q tile 4j+k at
            # col j*512 + k*128
            aT_sb = bigpool.tile([P, 2, S], BF16)
            # proj out: partitions 0:64 batch0 rows, 64:128 batch1 rows
            out_sb = bigpool.tile([P, 4, D], BF16)

            # ---- unit emitters ----
            def qk_unit(pair, st):
                """q,k projection for one pair, one s-slab (two 128-col tiles)."""
                xsl = xsl_list[st]
                for j in range(2):
                    ct = 2 * pair + j
                    ps = psQ.tile([P, 512], F32, tag="mm")
                    for d in range(DCH):
                        nc.tensor.matmul(
                            ps[:],
                            wqk_sb[:, d, ct * P:(ct + 1) * P],
                            xsl[:, d, :],
                            start=(d == 0),
                            stop=(d == DCH - 1),
                        )
                    nc.vector.tensor_scalar_add(
                        qkT_sb[:, ct, st * 512:(st + 1) * 512], ps[:], bqk_sb[:, ct:ct + 1]
                    )

            def v_unit(t16):
                """V projection for one 128-row s-tile (all 4 heads)."""
                st, sq = divmod(t16, 4)
                xsl = xsl_list[st]
                ps_full = psQ.tile([P, 512], F32, tag="mm", name="vacc")
                ps = ps_full[:, :HPC * HD]
                nc.tensor.matmul(ps[:], ones_sb[:, :], bv_sb[:, :], start=True, stop=False)
                for d in range(DCH):
                    nc.tensor.matmul(
                        ps[:],
                        xsl[:, d, sq * P:(sq + 1) * P],
                        wv_sb[:, d, :],
                        start=False,
                        stop=(d == DCH - 1),
                    )
                for h in range(HPC):
                    nc.vector.tensor_copy(
                        out=v1_sb[:, t16, h * 65:h * 65 + HD],
                        in_=ps[:, h * HD:(h + 1) * HD],
                    )

            # ---- attention group: scores/exp/PV with chunk-packed exp and
            # lag-2 PV.  The LAST 2 PV chunks and the normalize chain are NOT
            # emitted here: they return as carry closures that the next group
            # runs as its first fillers.  This removes the group-boundary
            # serialization (v2: PE waited ~2-4us at every boundary for
            # exp(last)->PV(last) before starting the next group's scores).
            def attn_group(pair, k, filler):
                chunks = attn_chunks(k)
                first_t = chunks[0][0][0]
                last_t = chunks[-1][-1][0]
                qvA = qkT_sb[0:HD, 2 * pair, :].rearrange("p (i g) -> p i g", g=512)
                qvB = qkT_sb[HD:P, 2 * pair, :].rearrange("p (i g) -> p i g", g=512)
                kv_ct = 2 * pair + 1
                pvA = psV.tile([P, 512], F32, tag="pv")
                pvB = psV.tile([P, 512], F32, tag="pv")

                def emit_pv(pr, ch):
                    for (t, off, N, s0) in ch:
                        for hh, pv in ((0, pvA), (1, pvB)):
                            h = 2 * pair + hh
                            nc.tensor.matmul(
                                pv[0:65, s0 * P:512],
                                v1_sb[:, t, h * 65:(h + 1) * 65],
                                pr[:, hh, off:off + N],
                                start=(t == first_t), stop=(t == last_t),
                            )

                pending = deque()
                for ci, ch in enumerate(chunks):
                    used = ch[-1][1] + ch[-1][2]
                    sc_full = psS.tile([P, 2, 512], F32, tag="sc")
                    for (t, off, N, s0) in ch:
                        sc = sc_full[:, :, off:off + N]
                        # diagonal tile: fold the causal mask into the PSUM
                        # accumulation (mask^T stationary x identity moving) so
                        # the score->exp path never touches the Vector queue
                        diag = t >= k and (t - k) % 4 == 0
                        nc.tensor.matmul(
                            sc[:, 0, :],
                            qkT_sb[0:HD, kv_ct, t * P:(t + 1) * P],
                            qvA[:, s0:4, k * P:(k + 1) * P],
                            start=True, stop=not diag, tile_position=(0, 0),
                        )
                        nc.tensor.matmul(
                            sc[:, 1, :],
                            qkT_sb[HD:P, kv_ct, t * P:(t + 1) * P],
                            qvB[:, s0:4, k * P:(k + 1) * P],
                            start=True, stop=not diag, tile_position=(64, 0),
                        )
                        if diag:
                            for hh in range(2):
                                nc.tensor.matmul(
                                    sc_full[:, hh, off:off + P],
                                    maskT_sb[:, :], ident_sb[:, :],
                                    start=False, stop=True,
                                )
                    pr = ppool.tile([P, 2, 512], BF16, tag="prob")
                    nc.scalar.activation(
                        pr[:, :, 0:used], sc_full[:, :, 0:used],
                        mybir.ActivationFunctionType.Exp, scale=SCALE,
                    )
                    if filler:
                        filler.popleft()()
                    if len(pending) >= 3:
                        emit_pv(*pending.popleft())
                    pending.append((pr, ch))
                while filler:
                    filler.popleft()()
                st = (pair, k, pvA, pvB)

                def mk_drain():
                    item = pending.popleft()
                    return lambda: emit_pv(*item)

                # carry: [drain PV x3, finish_a, finish_b]; popped at chunks
                # 0..4 of the next group, so the last drain trails exp(c_last)
                # by ~2 chunks of PE work and the bc matmul (finish_b) trails
                # the Vector reciprocal chain (finish_a) by one chunk
                cell = []
                carry = [
                    mk_drain(), mk_drain(), mk_drain(),
                    lambda: finish_a(st, cell),
                    lambda: finish_b(st, cell),
                ]
                return st, carry

            def finish_a(st, cell):
                # Vector-only half of the normalize: runs one filler slot
                # before finish_b so the bc matmul never waits on this chain
                pair, k, pvA, pvB = st
                sums = spool.tile([1, 2, 512], F32, tag="sums")
                nc.vector.tensor_copy(out=sums[:, 0, :], in_=pvA[64:65, :])
                nc.vector.tensor_copy(out=sums[:, 1, :], in_=pvB[64:65, :])
                # bf16 copies release the pv PSUM slots promptly
                pvc = spool.tile([HD, 2, 512], BF16, tag="pvc")
                nc.vector.tensor_copy(out=pvc[:, 0, :], in_=pvA[0:HD, :])
                nc.vector.tensor_copy(out=pvc[:, 1, :], in_=pvB[0:HD, :])
                rec = spool.tile([1, 2, 512], F32, tag="rec")
                nc.vector.reciprocal_approx_fast(rec[:], sums[:])
                recb = spool.tile([1, 2, 512], BF16, tag="recb")
                nc.vector.tensor_copy(out=recb[:], in_=rec[:])
                cell.append((pvc, recb))

            def finish_b(st, cell):
                pair, k, pvA, pvB = st
                pvc, recb = cell[0]
                for hh in range(2):
                    base = hh * HD
                    # broadcast 1/den across the 64 hd partitions with a K=1
                    # bf16 matmul (keeps GpSimd out of the normalize path)
                    bc = psQ.tile([P, 512], F32, tag="mm", name="bc")
                    nc.tensor.matmul(
                        bc[0:HD, :], ones_sb[:, 0:HD], recb[:, hh, :],
                        start=True, stop=True,
                    )
                    nc.vector.tensor_tensor(
                        out=aT_sb[base:base + HD, pair, :]
                        .rearrange("p (j q) -> p j q", q=4 * P)[:, :, k * P:(k + 1) * P],
                        in0=pvc[:, hh, :].rearrange("p (j f) -> p j f", f=P),
                        in1=bc[0:HD, :].rearrange("p (j f) -> p j f", f=P),
                        op=mybir.AluOpType.mult,
                    )

            def stage(k, buf, slot):
                # phase k: q tile 4j+k, 64-row half h -> dest core 2j+h
                for t0 in range(2):
                    for h in range(2):
                        nc.sync.dma_start(
                            out=buf[:, slot * 256 + t0 * P:slot * 256 + (t0 + 1) * P, :]
                            .rearrange("(j h) pp f -> j h pp f", h=2)[:, h]
                            .rearrange("j pp f -> pp j f"),
                            in_=aT_sb[:, t0, :]
                            .rearrange("pp (j q) -> pp j q", q=4 * P)
                            [:, :, k * P + 64 * h:k * P + 64 * h + 64],
                        )

            pjT_tiles = {}

            def pjT_unit(slot, buf, bslot, eng):
                pjT = pjpool.tile([P, DCH, P], BF16, tag="pjT", name=f"pjT{slot}")
                for t in range(2):
                    for b in range(2):
                        eng.dma_start(
                            out=pjT[:, :, b * 64:(b + 1) * 64]
                            .rearrange("pp (g t) f -> pp g t f", t=2)[:, :, t]
                            .rearrange("pp g f -> pp g f"),
                            in_=buf[4 * b:4 * b + 4,
                                    bslot * 256 + t * P:bslot * 256 + (t + 1) * P, :]
                            .rearrange("g pp f -> pp g f"),
                        )
                pjT_tiles[slot] = pjT

            def proj_unit(slot, dc, gate):
                pjT = pjT_tiles[slot]
                ps = psQ.tile([P, 512], F32, tag="mm", name="pacc")
                nc.tensor.matmul(
                    ps[:], gate[:, :],
                    bproj_sb[:, dc * 512:(dc + 1) * 512],
                    start=True, stop=False,
                )
                for ch in range(DCH):
                    nc.tensor.matmul(
                        ps[:],
                        pjT[:, ch, :],
                        wproj_sb[:, ch, dc * 512:(dc + 1) * 512],
                        start=False,
                        stop=(ch == DCH - 1),
                    )
                nc.vector.tensor_copy(out=out_sb[:, slot, dc * 512:(dc + 1) * 512], in_=ps[:])

            def out_unit(slot, dc):
                nc.sync.dma_start(
                    out=out_ext[slot, :, dc * 512:(dc + 1) * 512],
                    in_=out_sb[:, slot, dc * 512:(dc + 1) * 512],
                )

            # ---- emission schedule ----
            # E1: qk pair 0 only; pair 1 runs as fillers inside (0,0)/(0,1)
            # (pair-swapped first half), shrinking the serial head by ~14us.
            def qk_unit_single(st, j):
                xsl = xsl_list[st]
                ct = 2 + j
                ps = psQ.tile([P, 512], F32, tag="mm")
                for d in range(DCH):
                    nc.tensor.matmul(
                        ps[:],
                        wqk_sb[:, d, ct * P:(ct + 1) * P],
                        xsl[:, d, :],
                        start=(d == 0),
                        stop=(d == DCH - 1),
                    )
                nc.vector.tensor_scalar_add(
                    qkT_sb[:, ct, st * 512:(st + 1) * 512], ps[:], bqk_sb[:, ct:ct + 1]
                )

            for st in range(4):
                qk_unit(0, st)
            head_singles = [(0, 0), (0, 1)]
            for st, j in head_singles:
                qk_unit_single(st, j)
            # E2: V tiles 0..3 (needed by the first PV steps)
            for t16 in range(4):
                v_unit(t16)

            # E3: groups; each group's carry (last 3 PV drains + normalize)
            # runs as the NEXT group's first fillers, so neither the PE nor
            # the Scalar engine idles at group boundaries.
            # qk-p1 singles: 2 in the head (above), 2 in (0,0), 4 in (0,1);
            # V 4..13 fill (0,0) (deadline: own PV); V 14,15 -> (1,0).
            def mkv(a):
                return lambda: (v_unit(a), v_unit(a + 1))

            def mkq(st, j):
                return lambda: qk_unit_single(st, j)

            qfill = deque(
                mkq(st, j)
                for st in range(4)
                for j in range(2)
                if (st, j) not in head_singles
            )
            fill0 = deque(mkv(a) for a in range(4, 14, 2))
            fill0.append(qfill.popleft())
            fill0.append(qfill.popleft())

            G = [(0, 0), (0, 1), (1, 0), (1, 1), (0, 2), (1, 2), (0, 3), (1, 3)]
            carry = []
            fins = {}
            for i, (pair, k) in enumerate(G):
                filler = deque(carry)
                if i == 0:
                    filler.extend(fill0)
                if i == 1:
                    filler.extend(qfill)
                if i == 2:
                    filler.append(mkv(14))
                st_g, carry = attn_group(pair, k, filler)
                fins[(pair, k)] = st_g
                # staging + collectives as soon as each phase's finishes exist
                if (pair, k) == (1, 1):
                    stage(0, a2a_in1, 0)
                if (pair, k) == (0, 2):
                    stage(1, a2a_in1, 1)
                    nc.gpsimd.collective_compute(
                        "AllToAll", mybir.AluOpType.bypass,
                        ins=[a2a_in1[:].opt()], outs=[a2a_out1[:].opt()],
                        replica_groups=groups,
                    )
                    pjT_unit(0, a2a_out1, 0, nc.sync)
                    pjT_unit(1, a2a_out1, 1, nc.sync)
                if (pair, k) == (0, 3):
                    stage(2, a2a_in2, 0)
            # gate derives from fin(0,3)'s aT output (emitted inside (1,3)):
            # ready before the last group ends, so proj{0,1} can follow the
            # last PV drains with no Vector dependency in between.  The gate
            # still pins proj behind the attention stream in the PE queue,
            # so a slow A2A#1 cannot head-block anything earlier.
            gate_sb = wpool.tile([1, P], BF16)
            nc.vector.tensor_scalar(
                out=gate_sb[:], in0=aT_sb[0:1, 0, 384:512],
                scalar1=0.0, scalar2=1.0,
                op0=mybir.AluOpType.mult, op1=mybir.AluOpType.add,
            )
            proj_unit(0, 0, gate_sb)  # no exp dependency: runs immediately
            for fn in carry:  # drains + finish of (1,3)
                fn()
            stage(3, a2a_in2, 1)
            nc.gpsimd.collective_compute(
                "AllToAll", mybir.AluOpType.bypass,
                ins=[a2a_in2[:].opt()], outs=[a2a_out2[:].opt()],
                replica_groups=groups,
            )
            # pjT for A2A#2 slots on the Scalar queue: the sync queue carries
            # the out DMAs, which must not wait behind collective #2
            pjT_unit(2, a2a_out2, 0, nc.scalar)
            pjT_unit(3, a2a_out2, 1, nc.scalar)
            proj_unit(0, 1, gate_sb)
            out_unit(0, 0)
            out_unit(0, 1)
            for slot in (1, 2, 3):
                for dc in range(2):
                    proj_unit(slot, dc, gate_sb)
                    out_unit(slot, dc)

    nc.compile()
    return nc


def make_in_maps(x, w_qkv, b_qkv, w_proj, b_proj):
    import ml_dtypes

    bf16 = ml_dtypes.bfloat16
    x = np.asarray(x, dtype=np.float32)
    w_qkv = np.asarray(w_qkv, dtype=np.float32)
    b_qkv = np.asarray(b_qkv, dtype=np.float32)
    w_proj_bf = np.ascontiguousarray(np.asarray(w_proj, dtype=np.float32).astype(bf16))
    b_proj_bf = np.ascontiguousarray(
        np.asarray(b_proj, dtype=np.float32).astype(bf16).reshape(1, -1)
    )

    # maskT[q_local, kv_local] = 0 if q >= kv else NEG (stationary operand of
    # the mask matmul: out[kv, q] += maskT[q, kv] via identity moving data)
    mT = np.where(np.arange(P)[:, None] >= np.arange(P)[None, :], 0.0, NEG)
    ident = np.eye(P)
    mi = np.ascontiguousarray(
        np.concatenate([mT, ident], axis=1).astype(bf16)
    )

    in_maps = []
    for c in range(N_CORES):
        b, g = divmod(c, 4)
        hs = slice(g * HPC * HD, (g + 1) * HPC * HD)
        xT = np.ascontiguousarray(x[b].T.astype(bf16))           # [D, S]
        w_q = w_qkv[:, 0:D][:, hs]
        w_k = w_qkv[:, D:2 * D][:, hs]
        # columns: [q_p0 | k_p0 | q_p1 | k_p1]
        w_qk = np.ascontiguousarray(np.concatenate(
            [w_q[:, 0:128], w_k[:, 0:128], w_q[:, 128:256], w_k[:, 128:256]], axis=1
        ).astype(bf16))
        w_v = np.ascontiguousarray(w_qkv[:, 2 * D:3 * D][:, hs].astype(bf16))
        bq = b_qkv[0:D][hs]
        bk = b_qkv[D:2 * D][hs]
        bqk = np.stack([bq[0:128], bk[0:128], bq[128:256], bk[128:256]], axis=1)  # [128, 4]
        cst = np.ascontiguousarray(bqk.astype(np.float32))
        bv = np.ascontiguousarray(b_qkv[2 * D:3 * D][hs].reshape(1, -1).astype(bf16))
        in_maps.append(
            {
                "xT": xT,
                "w_qk": w_qk,
                "w_v": w_v,
                "consts": cst,
                "mi": mi,
                "b_v": bv,
                "w_proj": w_proj_bf,
                "b_proj": b_proj_bf,
            }
        )
    return in_maps


_NC_CACHE = None


def _install_ntff_shim():
    """Provide the antenv.axon_hooks module bass_utils wants for trace=True."""
    import sys as _sys
    import types

    if "antenv.axon_hooks" in _sys.modules:
        return
    try:
        from trn_agent_boot.trn_boot import _ntff_profile_via_ctypes

        hook = _ntff_profile_via_ctypes("/opt/axon/libaxon_pjrt.so")
    except Exception:
        hook = None
    mod = types.ModuleType("antenv.axon_hooks")
    mod._hook = hook
    mod.get_axon_ntff_profile_hook = lambda: mod._hook
    mod.set_axon_ntff_profile_hook = lambda h: setattr(mod, "_hook", h)
    _sys.modules["antenv.axon_hooks"] = mod


def kernel(x, w_qkv, b_qkv, w_proj, b_proj):
    global _NC_CACHE
    if _NC_CACHE is None:
        _NC_CACHE = build()
    nc = _NC_CACHE
    in_maps = make_in_maps(x, w_qkv, b_qkv, w_proj, b_proj)
    trace = bool(int(os.environ.get("KERNEL_TRACE", "0")))
    if trace:
        _install_ntff_shim()
    res = run_bass_kernel_spmd(
        nc,
        in_maps,
        core_ids=list(range(N_CORES)),
        trace=trace,
    )
    out = np.empty((B, S, D), dtype=np.float32)
    for c in range(N_CORES):
        oc = res.results[c]["out"]  # [4, 128, 1024] bf16
        j, h = divmod(c, 2)
        for k in range(4):
            r0 = 512 * j + 128 * k + 64 * h
            out[0, r0:r0 + 64, :] = oc[k, 0:64, :].astype(np.float32)
            out[1, r0:r0 + 64, :] = oc[k, 64:128, :].astype(np.float32)
    kernel.last_results = res
    return out


# revision 37
# speedup vs baseline: 1.0107x; 1.0107x over previous
"""Distributed causal multi-head attention for 8 Trainium2 NeuronCores.

Problem: B=2, S=2048, D=1024, H=16 heads (hd=64), fp32.
    qkv = x @ w_qkv + b_qkv ; causal softmax attention ; out = attn @ w_proj + b_proj

Distribution: core c -> (batch b = c//4, head group g = c%4 -> heads [4g, 4g+4)).
Transposed dataflow (channels on partitions, sequence on free axis); x arrives
host-transposed.

Restructured from the v1 phase-serial kernel (~245-257us) to ~215-225us.
Key findings from trace analysis, baked into this version:
  - k-phase output slots: each k-phase's attention output ships in 64-row
    "slots": receiver core c gets, per phase k, rows
    [512*(c//2) + 128*k + 64*(c%2), +64) of both batches.  A2A#1 carries
    phases {0,1} (doorbell ~mid-attention), A2A#2 carries {2,3} (doorbell at
    attention end).  Collectives cost ~15-25us of data movement PLUS ~27us
    of queue-release lag on the serializing GpSimd queue, and consumers see
    completion ~5us after the data phase ends; two A2As with all projection
    work at the end is the best overlap under those constants (3-4 smaller
    A2As cascade on the queue lag; 4-core-group A2As are unsupported - mesh
    needs >4 ranks).
  - A tiny warm-up AllGather issues at t~0: the FIRST collective pays
    ~55-60us extra firmware warm-up; this absorbs it off the critical path.
    Nothing else may use the GpSimd queue (it blocks behind collectives),
    so the V' ones-memset runs on Vector.
  - Projection packs batch0 rows on partitions 0:64 and batch1 on 64:128 so
    proj matmuls keep full 128-partition output despite 64-row slots.
  - The causal mask is applied INSIDE the score PSUM accumulation as an
    extra matmul (mask^T stationary x identity moving) on diagonal tiles:
    a Vector-engine mask add would queue behind the normalize chains and
    delay exp by up to ~3us per group (the Vector queue is in-order).
  - Each group's last 3 PV chunks and its normalize chain are carried into
    the NEXT group as its first 4 fillers, so neither the PE nor Scalar
    idles at group boundaries (boundary stalls also reset the PE p-state:
    it runs at 1.2GHz until ~3us of continuous execution, 2.4GHz after).
  - The softmax 1/sum broadcast matmul is bf16 (fp32 matmul is 4 cyc/row:
    853ns x16 = 13.6us of PE busy; bf16 is 213ns).
  - qk pair-1 and V-tile projections run as fillers inside the first two
    groups (pair-swapped order (0,0),(0,1),(1,0),(1,1),...), shrinking the
    serial pre-attention head.
  - pjT fetches for A2A#2 slots issue on the Scalar queue so they cannot
    head-block staging/out DMAs on the sync queue; pjT fetches for A2A#1
    prefetch on sync right after the collective.
  - Priority-ordered DMA with a small first chunk (wqk d0 + x slab0 d0);
    output ships as bf16 (host casts back to f32).
"""

import os
import sys
from collections import deque

sys.path.insert(0, "/opt/trn_rl_repo")

import numpy as np

import concourse.bass as bass
import concourse.tile as tile
from concourse import bacc, mybir
from concourse.bass_utils import run_bass_kernel_spmd

B, S, D = 2, 2048, 1024
H = 16
HD = 64
P = 128
N_CORES = 8
HPC = 4           # heads per core
DCH = D // P      # 8 contraction chunks
NQT = S // P      # 16 q tiles of 128
SCALE = 1.0 / 8.0  # 1/sqrt(hd)
NEG = -1.0e9

F32 = mybir.dt.float32
BF16 = mybir.dt.bfloat16


def attn_chunks(k):
    """Pack the kv-tile iterations of q-group k into <=512-col score chunks.

    First-fit: a late short tile (N=128) fills the slack of an earlier
    384-col chunk, so each chunk (= one exp call) is as full as possible."""
    T = 13 + k
    chunks = []  # [entries, used]
    for t in range(T):
        s0 = max(0, (t - k + 3) // 4)
        N = (4 - s0) * P
        for ch in chunks:
            if ch[1] + N <= 512:
                ch[0].append((t, ch[1], N, s0))
                ch[1] += N
                break
        else:
            chunks.append([[(t, 0, N, s0)], N])
    out = [c[0] for c in chunks]
    # split the final 512-col chunk into two halves: its exp call then
    # finishes sooner after its scores, so the next group's second score
    # chunk never stalls on the 2-deep PSUM score ring
    last = out[-1]
    if len(last) == 2:
        out[-1] = [last[0]]
        out.append([(last[1][0], 0, last[1][2], last[1][3])])
    return out


def build():
    nc = bacc.Bacc(num_devices=N_CORES)

    xT = nc.declare_dram_parameter("xT", [D, S], BF16, isOutput=False)
    # columns reordered host-side: [q_p0 | k_p0 | q_p1 | k_p1] (128 each)
    w_qk = nc.declare_dram_parameter("w_qk", [D, 2 * HPC * HD], BF16, isOutput=False)
    w_v = nc.declare_dram_parameter("w_v", [D, HPC * HD], BF16, isOutput=False)
    consts = nc.declare_dram_parameter("consts", [P, 4], F32, isOutput=False)
    # mi[:, 0:128] = mask^T (0 where q>=kv else NEG), mi[:, 128:256] = identity
    mi = nc.declare_dram_parameter("mi", [P, 2 * P], BF16, isOutput=False)
    b_v = nc.declare_dram_parameter("b_v", [1, HPC * HD], BF16, isOutput=False)
    w_proj = nc.declare_dram_parameter("w_proj", [D, D], BF16, isOutput=False)
    b_proj = nc.declare_dram_parameter("b_proj", [1, D], BF16, isOutput=False)
    # slot k: rows [512*(c//2) + 128*k + 64*(c%2), +64); partitions 0:64 = b0,
    # 64:128 = b1
    out_ext = nc.declare_dram_parameter("out", [4, P, D], BF16, isOutput=True)

    groups = [list(range(N_CORES))]

    with tile.TileContext(nc) as tc:
        with (
            tc.tile_pool(name="weights", bufs=1) as wpool,
            tc.tile_pool(name="xslab", bufs=4) as xpool,
            tc.tile_pool(name="qkT", bufs=1) as qkpool,
            tc.tile_pool(name="big", bufs=1) as bigpool,
            tc.tile_pool(name="prob", bufs=7) as ppool,
            tc.tile_pool(name="small", bufs=4) as spool,
            tc.tile_pool(name="pj", bufs=4) as pjpool,
            tc.tile_pool(name="dram", bufs=1, space="DRAM") as dpool,
            tc.tile_pool(name="psS", bufs=2, space="PSUM") as psS,   # scores 2 banks/slot
            tc.tile_pool(name="psV", bufs=2, space="PSUM") as psV,   # pv accumulators
            tc.tile_pool(name="psQ", bufs=2, space="PSUM") as psQ,   # qkv/proj groups
        ):
            # a2a layout: [dest core, bslot*256 + pair*128 + hd_part, 64 rows]
            # A2A#1 carries phases {0,1} (doorbell ~mid-attention; its ~27us
            # queue-release lag mostly clears before A2A#2's data exists);
            # A2A#2 carries phases {2,3} at attention end.
            a2a_in1 = dpool.tile([N_CORES, 512, 64], BF16, tag="a2a_in1")
            a2a_out1 = dpool.tile([N_CORES, 512, 64], BF16, tag="a2a_out1")
            a2a_in2 = dpool.tile([N_CORES, 512, 64], BF16, tag="a2a_in2")
            a2a_out2 = dpool.tile([N_CORES, 512, 64], BF16, tag="a2a_out2")
            dummy_in = dpool.tile([1, 16], BF16, tag="dummy_in")
            dummy_out = dpool.tile([N_CORES, 16], BF16, tag="dummy_out")

            # tiny warm-up collective issued immediately: the collectives
            # firmware has a large first-collective warm-up cost (observed:
            # the first real A2A would not move data before ~145us regardless
            # of when its inputs were staged); this absorbs it off the
            # critical path.
            nc.sync.dma_start(out=dummy_in[:], in_=b_v[:, 0:16])
            nc.gpsimd.collective_compute(
                "AllGather", mybir.AluOpType.bypass,
                ins=[dummy_in[:].opt()], outs=[dummy_out[:].opt()],
                replica_groups=groups,
            )

            # ---- DMA priority order ----
            wqk_sb = wpool.tile([P, DCH, 2 * HPC * HD], BF16)
            xsl_list = [
                xpool.tile([P, DCH, 512], BF16, tag="xslab", name=f"xsl{st}")
                for st in range(4)
            ]
            # smallest possible first chunks so the first matmul starts early
            nc.sync.dma_start(
                out=wqk_sb[:, 0:1, 0:256],
                in_=w_qk[:, 0:256].rearrange("(o p) c -> p o c", p=P)[:, 0:1, :],
            )
            nc.sync.dma_start(
                out=xsl_list[0][:, 0:1, :],
                in_=xT[:, :].rearrange("(o p) s -> p o s", p=P)[:, 0:1, 0:512],
            )
            consts_sb = wpool.tile([P, 4], F32)
            nc.sync.dma_start(out=consts_sb[:], in_=consts[:, :])
            nc.sync.dma_start(
                out=wqk_sb[:, 1:4, 0:256],
                in_=w_qk[:, 0:256].rearrange("(o p) c -> p o c", p=P)[:, 1:4, :],
            )
            nc.sync.dma_start(
                out=xsl_list[0][:, 1:4, :],
                in_=xT[:, :].rearrange("(o p) s -> p o s", p=P)[:, 1:4, 0:512],
            )
            nc.sync.dma_start(
                out=wqk_sb[:, 4:8, 0:256],
                in_=w_qk[:, 0:256].rearrange("(o p) c -> p o c", p=P)[:, 4:8, :],
            )
            nc.sync.dma_start(
                out=xsl_list[0][:, 4:8, :],
                in_=xT[:, :].rearrange("(o p) s -> p o s", p=P)[:, 4:8, 0:512],
            )
            mi_sb = wpool.tile([P, 2 * P], BF16)
            nc.sync.dma_start(out=mi_sb[:], in_=mi[:, :])
            nc.sync.dma_start(
                out=wqk_sb[:, :, 256:512],
                in_=w_qk[:, 256:512].rearrange("(o p) c -> p o c", p=P),
            )
            for st in range(1, 4):
                for dh in range(2):
                    dsl = slice(dh * 4, dh * 4 + 4)
                    nc.sync.dma_start(
                        out=xsl_list[st][:, dsl, :],
                        in_=xT[:, :].rearrange("(o p) s -> p o s", p=P)[:, dsl, st * 512:(st + 1) * 512],
                    )
            wv_sb = wpool.tile([P, DCH, HPC * HD], BF16)
            nc.sync.dma_start(out=wv_sb[:], in_=w_v[:, :].rearrange("(o p) c -> p o c", p=P))
            bv_sb = wpool.tile([1, HPC * HD], BF16)
            nc.sync.dma_start(out=bv_sb[:], in_=b_v[:, :])
            wproj_sb = wpool.tile([P, DCH, D], BF16)
            nc.sync.dma_start(out=wproj_sb[:], in_=w_proj[:, :].rearrange("(o p) c -> p o c", p=P))
            bproj_sb = wpool.tile([1, D], BF16)
            nc.sync.dma_start(out=bproj_sb[:], in_=b_proj[:, :])

            bqk_sb = consts_sb[:, 0:4]
            maskT_sb = mi_sb[:, 0:P]
            ident_sb = mi_sb[:, P:2 * P]
            ones_sb = wpool.tile([1, P], BF16)
            nc.vector.memset(ones_sb[:], 1.0)
            warm_sb = wpool.tile([1, 512], BF16)
            nc.vector.memset(warm_sb[:], 1.0)
            # ~10 throwaway matmuls: the PE clock needs ~3us of continuous
            # execution to ramp 0.65->2.4GHz; these run while the first
            # weight DMAs land so the real qk units start at speed
            for _ in range(20):
                wps = psQ.tile([P, 512], F32, tag="mm", name="warm")
                nc.tensor.matmul(
                    wps[:], ones_sb[:, :], warm_sb[:, :], start=True, stop=True,
                )

            # qkT layout: [128, ct, 2048]; ct: 0=q_p0, 1=k_p0, 2=q_p1, 3=k_p1
            qkT_sb = qkpool.tile([P, 4, S], BF16)
            # V': [128 kv_inner, 16 kv_outer, 4*65]; col 65h+64 = 1.0 (softmax denom)
            # memset on Vector, NOT GpSimd: the GpSimd queue must stay free
            # for collectives (the warm-up AllGather may occupy it for a
            # long time)
            v1_sb = bigpool.tile([P, NQT, HPC * 65], BF16)
            nc.vector.memset(v1_sb[:], 1.0)
            # attn outT: [128 (2 heads x 64), pair, 2048] bf16; q tile 4j+k at
            # col j*512 + k*128
            aT_sb = bigpool.tile([P, 2, S], BF16)
            # proj out: partitions 0:64 batch0 rows, 64:128 batch1 rows
            out_sb = bigpool.tile([P, 4, D], BF16)

            # ---- unit emitters ----
            def qk_unit(pair, st):
                """q,k projection for one pair, one s-slab (two 128-col tiles)."""
                xsl = xsl_list[st]
                for j in range(2):
                    ct = 2 * pair + j
                    ps = psQ.tile([P, 512], F32, tag="mm")
                    for d in range(DCH):
                        nc.tensor.matmul(
                            ps[:],
                            wqk_sb[:, d, ct * P:(ct + 1) * P],
                            xsl[:, d, :],
                            start=(d == 0),
                            stop=(d == DCH - 1),
                        )
                    nc.vector.tensor_scalar_add(
                        qkT_sb[:, ct, st * 512:(st + 1) * 512], ps[:], bqk_sb[:, ct:ct + 1]
                    )

            def v_unit(t16):
                """V projection for one 128-row s-tile (all 4 heads)."""
                st, sq = divmod(t16, 4)
                xsl = xsl_list[st]
                ps_full = psQ.tile([P, 512], F32, tag="mm", name="vacc")
                ps = ps_full[:, :HPC * HD]
                nc.tensor.matmul(ps[:], ones_sb[:, :], bv_sb[:, :], start=True, stop=False)
                for d in range(DCH):
                    nc.tensor.matmul(
                        ps[:],
                        xsl[:, d, sq * P:(sq + 1) * P],
                        wv_sb[:, d, :],
                        start=False,
                        stop=(d == DCH - 1),
                    )
                for h in range(HPC):
                    nc.vector.tensor_copy(
                        out=v1_sb[:, t16, h * 65:h * 65 + HD],
                        in_=ps[:, h * HD:(h + 1) * HD],
                    )

            # ---- attention group: scores/exp/PV with chunk-packed exp and
            # lag-2 PV.  The LAST 2 PV chunks and the normalize chain are NOT
            # emitted here: they return as carry closures that the next group
            # runs as its first fillers.  This removes the group-boundary
            # serialization (v2: PE waited ~2-4us at every boundary for
            # exp(last)->PV(last) before starting the next group's scores).
            def attn_group(pair, k, filler):
                chunks = attn_chunks(k)
                first_t = chunks[0][0][0]
                last_t = chunks[-1][-1][0]
                qvA = qkT_sb[0:HD, 2 * pair, :].rearrange("p (i g) -> p i g", g=512)
                qvB = qkT_sb[HD:P, 2 * pair, :].rearrange("p (i g) -> p i g", g=512)
                kv_ct = 2 * pair + 1
                pvA = psV.tile([P, 512], F32, tag="pv")
                pvB = psV.tile([P, 512], F32, tag="pv")

                def emit_pv(pr, ch):
                    for (t, off, N, s0) in ch:
                        for hh, pv in ((0, pvA), (1, pvB)):
                            h = 2 * pair + hh
                            nc.tensor.matmul(
                                pv[0:65, s0 * P:512],
                                v1_sb[:, t, h * 65:(h + 1) * 65],
                                pr[:, hh, off:off + N],
                                start=(t == first_t), stop=(t == last_t),
                            )

                pending = deque()
                for ci, ch in enumerate(chunks):
                    used = ch[-1][1] + ch[-1][2]
                    sc_full = psS.tile([P, 2, 512], F32, tag="sc")
                    for (t, off, N, s0) in ch:
                        sc = sc_full[:, :, off:off + N]
                        # diagonal tile: fold the causal mask into the PSUM
                        # accumulation (mask^T stationary x identity moving) so
                        # the score->exp path never touches the Vector queue
                        diag = t >= k and (t - k) % 4 == 0
                        nc.tensor.matmul(
                            sc[:, 0, :],
                            qkT_sb[0:HD, kv_ct, t * P:(t + 1) * P],
                            qvA[:, s0:4, k * P:(k + 1) * P],
                            start=True, stop=not diag, tile_position=(0, 0),
                        )
                        nc.tensor.matmul(
                            sc[:, 1, :],
                            qkT_sb[HD:P, kv_ct, t * P:(t + 1) * P],
                            qvB[:, s0:4, k * P:(k + 1) * P],
                            start=True, stop=not diag, tile_position=(64, 0),
                        )
                        if diag:
                            for hh in range(2):
                                nc.tensor.matmul(
                                    sc_full[:, hh, off:off + P],
                                    maskT_sb[:, :], ident_sb[:, :],
                                    start=False, stop=True,
                                )
                    pr = ppool.tile([P, 2, 512], BF16, tag="prob")
                    nc.scalar.activation(
                        pr[:, :, 0:used], sc_full[:, :, 0:used],
                        mybir.ActivationFunctionType.Exp, scale=SCALE,
                    )
                    if filler:
                        filler.popleft()()
                    if len(pending) >= 3:
                        emit_pv(*pending.popleft())
                    pending.append((pr, ch))
                while filler:
                    filler.popleft()()
                st = (pair, k, pvA, pvB)

                def mk_drain():
                    item = pending.popleft()
                    return lambda: emit_pv(*item)

                # carry: [drain PV x3, finish_a, finish_b]; popped at chunks
                # 0..4 of the next group, so the last drain trails exp(c_last)
                # by ~2 chunks of PE work and the bc matmul (finish_b) trails
                # the Vector reciprocal chain (finish_a) by one chunk
                cell = []
                carry = [
                    mk_drain(), mk_drain(), mk_drain(),
                    lambda: finish_a(st, cell),
                    lambda: finish_b(st, cell),
                ]
                return st, carry

            def finish_a(st, cell):
                # Vector-only half of the normalize: runs one filler slot
                # before finish_b so the bc matmul never waits on this chain
                pair, k, pvA, pvB = st
                sums = spool.tile([1, 2, 512], F32, tag="sums")
                nc.vector.tensor_copy(out=sums[:, 0, :], in_=pvA[64:65, :])
                nc.vector.tensor_copy(out=sums[:, 1, :], in_=pvB[64:65, :])
                # bf16 copies release the pv PSUM slots promptly
                pvc = spool.tile([HD, 2, 512], BF16, tag="pvc")
                nc.vector.tensor_copy(out=pvc[:, 0, :], in_=pvA[0:HD, :])
                nc.vector.tensor_copy(out=pvc[:, 1, :], in_=pvB[0:HD, :])
                rec = spool.tile([1, 2, 512], F32, tag="rec")
                nc.vector.reciprocal_approx_fast(rec[:], sums[:])
                recb = spool.tile([1, 2, 512], BF16, tag="recb")
                nc.vector.tensor_copy(out=recb[:], in_=rec[:])
                cell.append((pvc, recb))

            def finish_b(st, cell):
                pair, k, pvA, pvB = st
                pvc, recb = cell[0]
                for hh in range(2):
                    base = hh * HD
                    # broadcast 1/den across the 64 hd partitions with a K=1
                    # bf16 matmul (keeps GpSimd out of the normalize path)
                    bc = psQ.tile([P, 512], F32, tag="mm", name="bc")
                    nc.tensor.matmul(
                        bc[0:HD, :], ones_sb[:, 0:HD], recb[:, hh, :],
                        start=True, stop=True,
                    )
                    nc.vector.tensor_tensor(
                        out=aT_sb[base:base + HD, pair, :]
                        .rearrange("p (j q) -> p j q", q=4 * P)[:, :, k * P:(k + 1) * P],
                        in0=pvc[:, hh, :].rearrange("p (j f) -> p j f", f=P),
                        in1=bc[0:HD, :].rearrange("p (j f) -> p j f", f=P),
                        op=mybir.AluOpType.mult,
                    )

            def stage(k, buf, slot):
                # phase k: q tile 4j+k, 64-row half h -> dest core 2j+h
                for t0 in range(2):
                    for h in range(2):
                        nc.sync.dma_start(
                            out=buf[:, slot * 256 + t0 * P:slot * 256 + (t0 + 1) * P, :]
                            .rearrange("(j h) pp f -> j h pp f", h=2)[:, h]
                            .rearrange("j pp f -> pp j f"),
                            in_=aT_sb[:, t0, :]
                            .rearrange("pp (j q) -> pp j q", q=4 * P)
                            [:, :, k * P + 64 * h:k * P + 64 * h + 64],
                        )

            pjT_tiles = {}

            def pjT_unit(slot, buf, bslot, eng):
                pjT = pjpool.tile([P, DCH, P], BF16, tag="pjT", name=f"pjT{slot}")
                for t in range(2):
                    for b in range(2):
                        eng.dma_start(
                            out=pjT[:, :, b * 64:(b + 1) * 64]
                            .rearrange("pp (g t) f -> pp g t f", t=2)[:, :, t]
                            .rearrange("pp g f -> pp g f"),
                            in_=buf[4 * b:4 * b + 4,
                                    bslot * 256 + t * P:bslot * 256 + (t + 1) * P, :]
                            .rearrange("g pp f -> pp g f"),
                        )
                pjT_tiles[slot] = pjT

            def proj_unit(slot, dc, gate):
                pjT = pjT_tiles[slot]
                ps = psQ.tile([P, 512], F32, tag="mm", name="pacc")
                nc.tensor.matmul(
                    ps[:], gate[:, :],
                    bproj_sb[:, dc * 512:(dc + 1) * 512],
                    start=True, stop=False,
                )
                for ch in range(DCH):
                    nc.tensor.matmul(
                        ps[:],
                        pjT[:, ch, :],
                        wproj_sb[:, ch, dc * 512:(dc + 1) * 512],
                        start=False,
                        stop=(ch == DCH - 1),
                    )
                nc.vector.tensor_copy(out=out_sb[:, slot, dc * 512:(dc + 1) * 512], in_=ps[:])

            def out_unit(slot, dc):
                nc.sync.dma_start(
                    out=out_ext[slot, :, dc * 512:(dc + 1) * 512],
                    in_=out_sb[:, slot, dc * 512:(dc + 1) * 512],
                )

            # ---- emission schedule ----
            # E1: qk pair 0 only; pair 1 runs as fillers inside (0,0)/(0,1)
            # (pair-swapped first half), shrinking the serial head by ~14us.
            def qk_unit_single(st, j):
                xsl = xsl_list[st]
                ct = 2 + j
                ps = psQ.tile([P, 512], F32, tag="mm")
                for d in range(DCH):
                    nc.tensor.matmul(
                        ps[:],
                        wqk_sb[:, d, ct * P:(ct + 1) * P],
                        xsl[:, d, :],
                        start=(d == 0),
                        stop=(d == DCH - 1),
                    )
                nc.vector.tensor_scalar_add(
                    qkT_sb[:, ct, st * 512:(st + 1) * 512], ps[:], bqk_sb[:, ct:ct + 1]
                )

            for st in range(4):
                qk_unit(0, st)
            head_singles = [(0, 0), (0, 1)]
            for st, j in head_singles:
                qk_unit_single(st, j)
            # E2: V tiles 0..3 (needed by the first PV steps)
            for t16 in range(4):
                v_unit(t16)

            # E3: groups; each group's carry (last 3 PV drains + normalize)
            # runs as the NEXT group's first fillers, so neither the PE nor
            # the Scalar engine idles at group boundaries.
            # qk-p1 singles: 2 in the head (above), 2 in (0,0), 4 in (0,1);
            # V 4..13 fill (0,0) (deadline: own PV); V 14,15 -> (1,0).
            def mkv(a):
                return lambda: (v_unit(a), v_unit(a + 1))

            def mkq(st, j):
                return lambda: qk_unit_single(st, j)

            qfill = deque(
                mkq(st, j)
                for st in range(4)
                for j in range(2)
                if (st, j) not in head_singles
            )
            fill0 = deque(mkv(a) for a in range(4, 14, 2))
            fill0.append(qfill.popleft())
            fill0.append(qfill.popleft())

            G = [(0, 0), (0, 1), (1, 0), (1, 1), (0, 2), (1, 2), (0, 3), (1, 3)]
            carry = []
            fins = {}
            for i, (pair, k) in enumerate(G):
                filler = deque(carry)
                if i == 0:
                    filler.extend(fill0)
                if i == 1:
                    filler.extend(qfill)
                if i == 2:
                    filler.append(mkv(14))
                st_g, carry = attn_group(pair, k, filler)
                fins[(pair, k)] = st_g
                # staging + collectives as soon as each phase's finishes exist
                if (pair, k) == (1, 1):
                    stage(0, a2a_in1, 0)
                if (pair, k) == (0, 2):
                    stage(1, a2a_in1, 1)
                    nc.gpsimd.collective_compute(
                        "AllToAll", mybir.AluOpType.bypass,
                        ins=[a2a_in1[:].opt()], outs=[a2a_out1[:].opt()],
                        replica_groups=groups,
                    )
                    pjT_unit(0, a2a_out1, 0, nc.sync)
                    pjT_unit(1, a2a_out1, 1, nc.sync)
                if (pair, k) == (0, 3):
                    stage(2, a2a_in2, 0)
            # gate derives from fin(0,3)'s aT output (emitted inside (1,3)):
            # ready before the last group ends, so proj{0,1} can follow the
            # last PV drains with no Vector dependency in between.  The gate
            # still pins proj behind the attention stream in the PE queue,
            # so a slow A2A#1 cannot head-block anything earlier.
            gate_sb = wpool.tile([1, P], BF16)
            nc.vector.tensor_scalar(
                out=gate_sb[:], in0=aT_sb[0:1, 0, 384:512],
                scalar1=0.0, scalar2=1.0,
                op0=mybir.AluOpType.mult, op1=mybir.AluOpType.add,
            )
            proj_unit(0, 0, gate_sb)  # no exp dependency: runs immediately
            for fn in carry:  # drains + finish of (1,3)
                fn()
            stage(3, a2a_in2, 1)
            nc.gpsimd.collective_compute(
                "AllToAll", mybir.AluOpType.bypass,
                ins=[a2a_in2[:].opt()], outs=[a2a_out2[:].opt()],
                replica_groups=groups,
            )
            # pjT for A2A#2 slots on the Scalar queue: the sync queue carries
            # the out DMAs, which must not wait behind collective #2
            pjT_unit(2, a2a_out2, 0, nc.scalar)
            pjT_unit(3, a2a_out2, 1, nc.scalar)
            proj_unit(0, 1, gate_sb)
            out_unit(0, 0)
            out_unit(0, 1)
            for slot in (1, 2, 3):
                for dc in range(2):
                    proj_unit(slot, dc, gate_sb)
                    out_unit(slot, dc)

    nc.compile()
    return nc


def make_in_maps(x, w_qkv, b_qkv, w_proj, b_proj):
    import ml_dtypes

    bf16 = ml_dtypes.bfloat16
    x = np.asarray(x, dtype=np.float32)
    w_qkv = np.asarray(w_qkv, dtype=np.float32)
    b_qkv = np.asarray(b_qkv, dtype=np.float32)
    w_proj_bf = np.ascontiguousarray(np.asarray(w_proj, dtype=np.float32).astype(bf16))
    b_proj_bf = np.ascontiguousarray(
        np.asarray(b_proj, dtype=np.float32).astype(bf16).reshape(1, -1)
    )

    # maskT[q_local, kv_local] = 0 if q >= kv else NEG (stationary operand of
    # the mask matmul: out[kv, q] += maskT[q, kv] via identity moving data)
    mT = np.where(np.arange(P)[:, None] >= np.arange(P)[None, :], 0.0, NEG)
    ident = np.eye(P)
    mi = np.ascontiguousarray(
        np.concatenate([mT, ident], axis=1).astype(bf16)
    )

    in_maps = []
    for c in range(N_CORES):
        b, g = divmod(c, 4)
        hs = slice(g * HPC * HD, (g + 1) * HPC * HD)
        xT = np.ascontiguousarray(x[b].T.astype(bf16))           # [D, S]
        w_q = w_qkv[:, 0:D][:, hs]
        w_k = w_qkv[:, D:2 * D][:, hs]
        # columns: [q_p0 | k_p0 | q_p1 | k_p1]
        w_qk = np.ascontiguousarray(np.concatenate(
            [w_q[:, 0:128], w_k[:, 0:128], w_q[:, 128:256], w_k[:, 128:256]], axis=1
        ).astype(bf16))
        w_v = np.ascontiguousarray(w_qkv[:, 2 * D:3 * D][:, hs].astype(bf16))
        bq = b_qkv[0:D][hs]
        bk = b_qkv[D:2 * D][hs]
        bqk = np.stack([bq[0:128], bk[0:128], bq[128:256], bk[128:256]], axis=1)  # [128, 4]
        cst = np.ascontiguousarray(bqk.astype(np.float32))
        bv = np.ascontiguousarray(b_qkv[2 * D:3 * D][hs].reshape(1, -1).astype(bf16))
        in_maps.append(
            {
                "xT": xT,
                "w_qk": w_qk,
                "w_v": w_v,
                "consts": cst,
                "mi": mi,
                "b_v": bv,
                "w_proj": w_proj_bf,
                "b_proj": b_proj_bf,
            }
        )
    return in_maps


_NC_CACHE = None


def _install_ntff_shim():
    """Provide the antenv.axon_hooks module bass_utils wants for trace=True."""
    import sys as _sys
    import types

    if "antenv.axon_hooks" in _sys.modules:
        return
    try:
        from trn_agent_boot.trn_boot import _ntff_profile_via_ctypes

        hook = _ntff_profile_via_ctypes("/opt/axon/libaxon_pjrt.so")
    except Exception:
        hook = None
    mod = types.ModuleType("antenv.axon_hooks")
    mod._hook = hook
    mod.get_axon_ntff_profile_hook = lambda: mod._hook
    mod.set_axon_ntff_profile_hook = lambda h: setattr(mod, "_hook", h)
    _sys.modules["antenv.axon_hooks"] = mod


def kernel(x, w_qkv, b_qkv, w_proj, b_proj):
    global _NC_CACHE
    if _NC_CACHE is None:
        _NC_CACHE = build()
    nc = _NC_CACHE
    in_maps = make_in_maps(x, w_qkv, b_qkv, w_proj, b_proj)
    trace = bool(int(os.environ.get("KERNEL_TRACE", "0")))
    if trace:
        _install_ntff_shim()
    res = run_bass_kernel_spmd(
        nc,
        in_maps,
        core_ids=list(range(N_CORES)),
        trace=trace,
    )
    out = np.empty((B, S, D), dtype=np.float32)
    for c in range(N_CORES):
        oc = res.results[c]["out"]  # [4, 128, 1024] bf16
        j, h = divmod(c, 2)
        for k in range(4):
            r0 = 512 * j + 128 * k + 64 * h
            out[0, r0:r0 + 64, :] = oc[k, 0:64, :].astype(np.float32)
            out[1, r0:r0 + 64, :] = oc[k, 64:128, :].astype(np.float32)
    kernel.last_results = res
    return out


# revision 38
# speedup vs baseline: 1.0550x; 1.0438x over previous
"""Distributed causal multi-head attention for 8 Trainium2 NeuronCores.

Problem: B=2, S=2048, D=1024, H=16 heads (hd=64), fp32.
    qkv = x @ w_qkv + b_qkv ; causal softmax attention ; out = attn @ w_proj + b_proj

Distribution: core c -> (batch b = c//4, head group g = c%4 -> heads [4g, 4g+4)).
Transposed dataflow (channels on partitions, sequence on free axis); x arrives
host-transposed.

Restructured from the v1 phase-serial kernel (~245-257us) to ~215-225us.
Key findings from trace analysis, baked into this version:
  - k-phase output slots: each k-phase's attention output ships in 64-row
    "slots": receiver core c gets, per phase k, rows
    [512*(c//2) + 128*k + 64*(c%2), +64) of both batches.  A2A#1 carries
    phases {0,1} (doorbell ~mid-attention), A2A#2 carries {2,3} (doorbell at
    attention end).  Collectives cost ~15-25us of data movement PLUS ~27us
    of queue-release lag on the serializing GpSimd queue, and consumers see
    completion ~5us after the data phase ends; two A2As with all projection
    work at the end is the best overlap under those constants (3-4 smaller
    A2As cascade on the queue lag; 4-core-group A2As are unsupported - mesh
    needs >4 ranks).
  - A tiny warm-up AllGather issues at t~0: the FIRST collective pays
    ~55-60us extra firmware warm-up; this absorbs it off the critical path.
    Nothing else may use the GpSimd queue (it blocks behind collectives),
    so the V' ones-memset runs on Vector.
  - Projection packs batch0 rows on partitions 0:64 and batch1 on 64:128 so
    proj matmuls keep full 128-partition output despite 64-row slots.
  - The causal mask is applied INSIDE the score PSUM accumulation as an
    extra matmul (mask^T stationary x identity moving) on diagonal tiles:
    a Vector-engine mask add would queue behind the normalize chains and
    delay exp by up to ~3us per group (the Vector queue is in-order).
  - Each group's last 3 PV chunks and its normalize chain are carried into
    the NEXT group as its first 4 fillers, so neither the PE nor Scalar
    idles at group boundaries (boundary stalls also reset the PE p-state:
    it runs at 1.2GHz until ~3us of continuous execution, 2.4GHz after).
  - The softmax 1/sum broadcast matmul is bf16 (fp32 matmul is 4 cyc/row:
    853ns x16 = 13.6us of PE busy; bf16 is 213ns).
  - qk pair-1 and V-tile projections run as fillers inside the first two
    groups (pair-swapped order (0,0),(0,1),(1,0),(1,1),...), shrinking the
    serial pre-attention head.
  - pjT fetches for A2A#2 slots issue on the Scalar queue so they cannot
    head-block staging/out DMAs on the sync queue; pjT fetches for A2A#1
    prefetch on sync right after the collective.
  - Priority-ordered DMA with a small first chunk (wqk d0 + x slab0 d0);
    output ships as bf16 (host casts back to f32).
"""

import os
import sys
from collections import deque

sys.path.insert(0, "/opt/trn_rl_repo")

import numpy as np

import concourse.bass as bass
import concourse.tile as tile
from concourse import bacc, mybir
from concourse.bass_utils import run_bass_kernel_spmd

B, S, D = 2, 2048, 1024
H = 16
HD = 64
P = 128
N_CORES = 8
HPC = 4           # heads per core
DCH = D // P      # 8 contraction chunks
NQT = S // P      # 16 q tiles of 128
SCALE = 1.0 / 8.0  # 1/sqrt(hd)
NEG = -1.0e9

F32 = mybir.dt.float32
BF16 = mybir.dt.bfloat16


def attn_chunks(k):
    """Pack the kv-tile iterations of q-group k into <=512-col score chunks.

    First-fit: a late short tile (N=128) fills the slack of an earlier
    384-col chunk, so each chunk (= one exp call) is as full as possible."""
    T = 13 + k
    chunks = []  # [entries, used]
    for t in range(T):
        s0 = max(0, (t - k + 3) // 4)
        N = (4 - s0) * P
        for ch in chunks:
            if ch[1] + N <= 512:
                ch[0].append((t, ch[1], N, s0))
                ch[1] += N
                break
        else:
            chunks.append([[(t, 0, N, s0)], N])
    out = [c[0] for c in chunks]
    # split the final 512-col chunk into two halves: its exp call then
    # finishes sooner after its scores, so the next group's second score
    # chunk never stalls on the 2-deep PSUM score ring
    last = out[-1]
    if len(last) == 2:
        out[-1] = [last[0]]
        out.append([(last[1][0], 0, last[1][2], last[1][3])])
    return out


def build():
    nc = bacc.Bacc(num_devices=N_CORES)

    xT = nc.declare_dram_parameter("xT", [D, S], BF16, isOutput=False)
    # columns reordered host-side: [q_p0 | k_p0 | q_p1 | k_p1] (128 each)
    w_qk = nc.declare_dram_parameter("w_qk", [D, 2 * HPC * HD], BF16, isOutput=False)
    w_v = nc.declare_dram_parameter("w_v", [D, HPC * HD], BF16, isOutput=False)
    consts = nc.declare_dram_parameter("consts", [P, 4], F32, isOutput=False)
    # mi[:, 0:128] = mask^T (0 where q>=kv else NEG), mi[:, 128:256] = identity
    mi = nc.declare_dram_parameter("mi", [P, 2 * P], BF16, isOutput=False)
    b_v = nc.declare_dram_parameter("b_v", [1, HPC * HD], BF16, isOutput=False)
    w_proj = nc.declare_dram_parameter("w_proj", [D, D], BF16, isOutput=False)
    b_proj = nc.declare_dram_parameter("b_proj", [1, D], BF16, isOutput=False)
    # slot k: rows [512*(c//2) + 128*k + 64*(c%2), +64); partitions 0:64 = b0,
    # 64:128 = b1
    out_ext = nc.declare_dram_parameter("out", [4, P, D], BF16, isOutput=True)

    groups = [list(range(N_CORES))]

    with tile.TileContext(nc) as tc:
        with (
            tc.tile_pool(name="weights", bufs=1) as wpool,
            tc.tile_pool(name="xslab", bufs=4) as xpool,
            tc.tile_pool(name="qkT", bufs=1) as qkpool,
            tc.tile_pool(name="big", bufs=1) as bigpool,
            tc.tile_pool(name="prob", bufs=7) as ppool,
            tc.tile_pool(name="small", bufs=4) as spool,
            tc.tile_pool(name="pj", bufs=4) as pjpool,
            tc.tile_pool(name="dram", bufs=1, space="DRAM") as dpool,
            tc.tile_pool(name="psS", bufs=2, space="PSUM") as psS,   # scores 2 banks/slot
            tc.tile_pool(name="psV", bufs=2, space="PSUM") as psV,   # pv accumulators
            tc.tile_pool(name="psQ", bufs=2, space="PSUM") as psQ,   # qkv/proj groups
        ):
            # a2a layout: [dest core, bslot*256 + pair*128 + hd_part, 64 rows]
            # A2A#1 carries phases {0,1} (doorbell ~mid-attention; its ~27us
            # queue-release lag mostly clears before A2A#2's data exists);
            # A2A#2 carries phases {2,3} at attention end.
            a2a_in1 = dpool.tile([N_CORES, 512, 64], BF16, tag="a2a_in1")
            a2a_out1 = dpool.tile([N_CORES, 512, 64], BF16, tag="a2a_out1")
            a2a_in2 = dpool.tile([N_CORES, 512, 64], BF16, tag="a2a_in2")
            a2a_out2 = dpool.tile([N_CORES, 512, 64], BF16, tag="a2a_out2")
            dummy_in = dpool.tile([1, 16], BF16, tag="dummy_in")
            dummy_out = dpool.tile([N_CORES, 16], BF16, tag="dummy_out")

            # tiny warm-up collective issued immediately: the collectives
            # firmware has a large first-collective warm-up cost (observed:
            # the first real A2A would not move data before ~145us regardless
            # of when its inputs were staged); this absorbs it off the
            # critical path.
            nc.sync.dma_start(out=dummy_in[:], in_=b_v[:, 0:16])
            nc.gpsimd.collective_compute(
                "AllGather", mybir.AluOpType.bypass,
                ins=[dummy_in[:].opt()], outs=[dummy_out[:].opt()],
                replica_groups=groups,
            )

            # ---- DMA priority order ----
            wqk_sb = wpool.tile([P, DCH, 2 * HPC * HD], BF16)
            xsl_list = [
                xpool.tile([P, DCH, 512], BF16, tag="xslab", name=f"xsl{st}")
                for st in range(4)
            ]
            # smallest possible first chunks so the first matmul starts early
            nc.sync.dma_start(
                out=wqk_sb[:, 0:1, 0:256],
                in_=w_qk[:, 0:256].rearrange("(o p) c -> p o c", p=P)[:, 0:1, :],
            )
            nc.sync.dma_start(
                out=xsl_list[0][:, 0:1, :],
                in_=xT[:, :].rearrange("(o p) s -> p o s", p=P)[:, 0:1, 0:512],
            )
            consts_sb = wpool.tile([P, 4], F32)
            nc.sync.dma_start(out=consts_sb[:], in_=consts[:, :])
            nc.sync.dma_start(
                out=wqk_sb[:, 1:4, 0:256],
                in_=w_qk[:, 0:256].rearrange("(o p) c -> p o c", p=P)[:, 1:4, :],
            )
            nc.sync.dma_start(
                out=xsl_list[0][:, 1:4, :],
                in_=xT[:, :].rearrange("(o p) s -> p o s", p=P)[:, 1:4, 0:512],
            )
            nc.sync.dma_start(
                out=wqk_sb[:, 4:8, 0:256],
                in_=w_qk[:, 0:256].rearrange("(o p) c -> p o c", p=P)[:, 4:8, :],
            )
            nc.sync.dma_start(
                out=xsl_list[0][:, 4:8, :],
                in_=xT[:, :].rearrange("(o p) s -> p o s", p=P)[:, 4:8, 0:512],
            )
            mi_sb = wpool.tile([P, 2 * P], BF16)
            nc.sync.dma_start(out=mi_sb[:], in_=mi[:, :])
            nc.sync.dma_start(
                out=wqk_sb[:, :, 256:512],
                in_=w_qk[:, 256:512].rearrange("(o p) c -> p o c", p=P),
            )
            for st in range(1, 4):
                for dh in range(2):
                    dsl = slice(dh * 4, dh * 4 + 4)
                    nc.sync.dma_start(
                        out=xsl_list[st][:, dsl, :],
                        in_=xT[:, :].rearrange("(o p) s -> p o s", p=P)[:, dsl, st * 512:(st + 1) * 512],
                    )
            wv_sb = wpool.tile([P, DCH, HPC * HD], BF16)
            nc.sync.dma_start(out=wv_sb[:], in_=w_v[:, :].rearrange("(o p) c -> p o c", p=P))
            bv_sb = wpool.tile([1, HPC * HD], BF16)
            nc.sync.dma_start(out=bv_sb[:], in_=b_v[:, :])
            wproj_sb = wpool.tile([P, DCH, D], BF16)
            nc.sync.dma_start(out=wproj_sb[:], in_=w_proj[:, :].rearrange("(o p) c -> p o c", p=P))
            bproj_sb = wpool.tile([1, D], BF16)
            nc.sync.dma_start(out=bproj_sb[:], in_=b_proj[:, :])

            bqk_sb = consts_sb[:, 0:4]
            maskT_sb = mi_sb[:, 0:P]
            ident_sb = mi_sb[:, P:2 * P]
            ones_sb = wpool.tile([1, P], BF16)
            nc.vector.memset(ones_sb[:], 1.0)
            warm_sb = wpool.tile([1, 512], BF16)
            nc.vector.memset(warm_sb[:], 1.0)
            # ~10 throwaway matmuls: the PE clock needs ~3us of continuous
            # execution to ramp 0.65->2.4GHz; these run while the first
            # weight DMAs land so the real qk units start at speed
            for _ in range(10):
                wps = psQ.tile([P, 512], F32, tag="mm", name="warm")
                nc.tensor.matmul(
                    wps[:], ones_sb[:, :], warm_sb[:, :], start=True, stop=True,
                )

            # qkT layout: [128, ct, 2048]; ct: 0=q_p0, 1=k_p0, 2=q_p1, 3=k_p1
            qkT_sb = qkpool.tile([P, 4, S], BF16)
            # V': [128 kv_inner, 16 kv_outer, 4*65]; col 65h+64 = 1.0 (softmax denom)
            # memset on Vector, NOT GpSimd: the GpSimd queue must stay free
            # for collectives (the warm-up AllGather may occupy it for a
            # long time)
            v1_sb = bigpool.tile([P, NQT, HPC * 65], BF16)
            nc.vector.memset(v1_sb[:], 1.0)
            # attn outT: [128 (2 heads x 64), pair, 2048] bf16; q tile 4j+k at
            # col j*512 + k*128
            aT_sb = bigpool.tile([P, 2, S], BF16)
            # proj out: partitions 0:64 batch0 rows, 64:128 batch1 rows
            out_sb = bigpool.tile([P, 4, D], BF16)

            # ---- unit emitters ----
            def qk_unit(pair, st):
                """q,k projection for one pair, one s-slab (two 128-col tiles)."""
                xsl = xsl_list[st]
                for j in range(2):
                    ct = 2 * pair + j
                    ps = psQ.tile([P, 512], F32, tag="mm")
                    for d in range(DCH):
                        nc.tensor.matmul(
                            ps[:],
                            wqk_sb[:, d, ct * P:(ct + 1) * P],
                            xsl[:, d, :],
                            start=(d == 0),
                            stop=(d == DCH - 1),
                        )
                    nc.vector.tensor_scalar_add(
                        qkT_sb[:, ct, st * 512:(st + 1) * 512], ps[:], bqk_sb[:, ct:ct + 1]
                    )

            def v_unit(t16):
                """V projection for one 128-row s-tile (all 4 heads)."""
                st, sq = divmod(t16, 4)
                xsl = xsl_list[st]
                ps_full = psQ.tile([P, 512], F32, tag="mm", name="vacc")
                ps = ps_full[:, :HPC * HD]
                nc.tensor.matmul(ps[:], ones_sb[:, :], bv_sb[:, :], start=True, stop=False)
                for d in range(DCH):
                    nc.tensor.matmul(
                        ps[:],
                        xsl[:, d, sq * P:(sq + 1) * P],
                        wv_sb[:, d, :],
                        start=False,
                        stop=(d == DCH - 1),
                    )
                for h in range(HPC):
                    nc.vector.tensor_copy(
                        out=v1_sb[:, t16, h * 65:h * 65 + HD],
                        in_=ps[:, h * HD:(h + 1) * HD],
                    )

            # ---- attention group: scores/exp/PV with chunk-packed exp and
            # lag-2 PV.  The LAST 2 PV chunks and the normalize chain are NOT
            # emitted here: they return as carry closures that the next group
            # runs as its first fillers.  This removes the group-boundary
            # serialization (v2: PE waited ~2-4us at every boundary for
            # exp(last)->PV(last) before starting the next group's scores).
            def attn_group(pair, k, filler):
                chunks = attn_chunks(k)
                first_t = chunks[0][0][0]
                last_t = chunks[-1][-1][0]
                qvA = qkT_sb[0:HD, 2 * pair, :].rearrange("p (i g) -> p i g", g=512)
                qvB = qkT_sb[HD:P, 2 * pair, :].rearrange("p (i g) -> p i g", g=512)
                kv_ct = 2 * pair + 1
                pvA = psV.tile([P, 512], F32, tag="pv")
                pvB = psV.tile([P, 512], F32, tag="pv")

                def emit_pv(pr, ch):
                    for (t, off, N, s0) in ch:
                        for hh, pv in ((0, pvA), (1, pvB)):
                            h = 2 * pair + hh
                            nc.tensor.matmul(
                                pv[0:65, s0 * P:512],
                                v1_sb[:, t, h * 65:(h + 1) * 65],
                                pr[:, hh, off:off + N],
                                start=(t == first_t), stop=(t == last_t),
                            )

                pending = deque()
                for ci, ch in enumerate(chunks):
                    used = ch[-1][1] + ch[-1][2]
                    sc_full = psS.tile([P, 2, 512], F32, tag="sc")
                    for (t, off, N, s0) in ch:
                        sc = sc_full[:, :, off:off + N]
                        # diagonal tile: fold the causal mask into the PSUM
                        # accumulation (mask^T stationary x identity moving) so
                        # the score->exp path never touches the Vector queue
                        diag = t >= k and (t - k) % 4 == 0
                        nc.tensor.matmul(
                            sc[:, 0, :],
                            qkT_sb[0:HD, kv_ct, t * P:(t + 1) * P],
                            qvA[:, s0:4, k * P:(k + 1) * P],
                            start=True, stop=not diag, tile_position=(0, 0),
                        )
                        nc.tensor.matmul(
                            sc[:, 1, :],
                            qkT_sb[HD:P, kv_ct, t * P:(t + 1) * P],
                            qvB[:, s0:4, k * P:(k + 1) * P],
                            start=True, stop=not diag, tile_position=(64, 0),
                        )
                        if diag:
                            for hh in range(2):
                                nc.tensor.matmul(
                                    sc_full[:, hh, off:off + P],
                                    maskT_sb[:, :], ident_sb[:, :],
                                    start=False, stop=True,
                                )
                    pr = ppool.tile([P, 2, 512], BF16, tag="prob")
                    nc.scalar.activation(
                        pr[:, :, 0:used], sc_full[:, :, 0:used],
                        mybir.ActivationFunctionType.Exp, scale=SCALE,
                    )
                    if filler:
                        filler.popleft()()
                    if len(pending) >= 3:
                        emit_pv(*pending.popleft())
                    pending.append((pr, ch))
                while filler:
                    filler.popleft()()
                st = (pair, k, pvA, pvB)

                def mk_drain():
                    item = pending.popleft()
                    return lambda: emit_pv(*item)

                # carry: [drain PV x3, finish_a, finish_b]; popped at chunks
                # 0..4 of the next group, so the last drain trails exp(c_last)
                # by ~2 chunks of PE work and the bc matmul (finish_b) trails
                # the Vector reciprocal chain (finish_a) by one chunk
                cell = []
                carry = [
                    mk_drain(), mk_drain(), mk_drain(),
                    lambda: finish_a(st, cell),
                    lambda: finish_b(st, cell),
                ]
                return st, carry

            def finish_a(st, cell):
                # Vector-only half of the normalize: runs one filler slot
                # before finish_b so the bc matmul never waits on this chain
                pair, k, pvA, pvB = st
                sums = spool.tile([1, 2, 512], F32, tag="sums")
                nc.vector.tensor_copy(out=sums[:, 0, :], in_=pvA[64:65, :])
                nc.vector.tensor_copy(out=sums[:, 1, :], in_=pvB[64:65, :])
                # bf16 copies release the pv PSUM slots promptly
                pvc = spool.tile([HD, 2, 512], BF16, tag="pvc")
                nc.vector.tensor_copy(out=pvc[:, 0, :], in_=pvA[0:HD, :])
                nc.vector.tensor_copy(out=pvc[:, 1, :], in_=pvB[0:HD, :])
                rec = spool.tile([1, 2, 512], F32, tag="rec")
                nc.vector.reciprocal_approx_fast(rec[:], sums[:])
                recb = spool.tile([1, 2, 512], BF16, tag="recb")
                nc.vector.tensor_copy(out=recb[:], in_=rec[:])
                cell.append((pvc, recb))

            def finish_b(st, cell):
                pair, k, pvA, pvB = st
                pvc, recb = cell[0]
                for hh in range(2):
                    base = hh * HD
                    # broadcast 1/den across the 64 hd partitions with a K=1
                    # bf16 matmul (keeps GpSimd out of the normalize path)
                    bc = psQ.tile([P, 512], F32, tag="mm", name="bc")
                    nc.tensor.matmul(
                        bc[0:HD, :], ones_sb[:, 0:HD], recb[:, hh, :],
                        start=True, stop=True,
                    )
                    nc.vector.tensor_tensor(
                        out=aT_sb[base:base + HD, pair, :]
                        .rearrange("p (j q) -> p j q", q=4 * P)[:, :, k * P:(k + 1) * P],
                        in0=pvc[:, hh, :].rearrange("p (j f) -> p j f", f=P),
                        in1=bc[0:HD, :].rearrange("p (j f) -> p j f", f=P),
                        op=mybir.AluOpType.mult,
                    )

            def stage(k, buf, slot):
                # phase k: q tile 4j+k, 64-row half h -> dest core 2j+h
                for t0 in range(2):
                    for h in range(2):
                        nc.sync.dma_start(
                            out=buf[:, slot * 256 + t0 * P:slot * 256 + (t0 + 1) * P, :]
                            .rearrange("(j h) pp f -> j h pp f", h=2)[:, h]
                            .rearrange("j pp f -> pp j f"),
                            in_=aT_sb[:, t0, :]
                            .rearrange("pp (j q) -> pp j q", q=4 * P)
                            [:, :, k * P + 64 * h:k * P + 64 * h + 64],
                        )

            pjT_tiles = {}

            def pjT_unit(slot, buf, bslot, eng):
                pjT = pjpool.tile([P, DCH, P], BF16, tag="pjT", name=f"pjT{slot}")
                for t in range(2):
                    for b in range(2):
                        eng.dma_start(
                            out=pjT[:, :, b * 64:(b + 1) * 64]
                            .rearrange("pp (g t) f -> pp g t f", t=2)[:, :, t]
                            .rearrange("pp g f -> pp g f"),
                            in_=buf[4 * b:4 * b + 4,
                                    bslot * 256 + t * P:bslot * 256 + (t + 1) * P, :]
                            .rearrange("g pp f -> pp g f"),
                        )
                pjT_tiles[slot] = pjT

            def proj_unit(slot, dc, gate):
                pjT = pjT_tiles[slot]
                ps = psQ.tile([P, 512], F32, tag="mm", name="pacc")
                nc.tensor.matmul(
                    ps[:], gate[:, :],
                    bproj_sb[:, dc * 512:(dc + 1) * 512],
                    start=True, stop=False,
                )
                for ch in range(DCH):
                    nc.tensor.matmul(
                        ps[:],
                        pjT[:, ch, :],
                        wproj_sb[:, ch, dc * 512:(dc + 1) * 512],
                        start=False,
                        stop=(ch == DCH - 1),
                    )
                nc.vector.tensor_copy(out=out_sb[:, slot, dc * 512:(dc + 1) * 512], in_=ps[:])

            def out_unit(slot, dc):
                nc.sync.dma_start(
                    out=out_ext[slot, :, dc * 512:(dc + 1) * 512],
                    in_=out_sb[:, slot, dc * 512:(dc + 1) * 512],
                )

            # ---- emission schedule ----
            # E1: qk pair 0 only; pair 1 runs as fillers inside (0,0)/(0,1)
            # (pair-swapped first half), shrinking the serial head by ~14us.
            def qk_unit_single(st, j):
                xsl = xsl_list[st]
                ct = 2 + j
                ps = psQ.tile([P, 512], F32, tag="mm")
                for d in range(DCH):
                    nc.tensor.matmul(
                        ps[:],
                        wqk_sb[:, d, ct * P:(ct + 1) * P],
                        xsl[:, d, :],
                        start=(d == 0),
                        stop=(d == DCH - 1),
                    )
                nc.vector.tensor_scalar_add(
                    qkT_sb[:, ct, st * 512:(st + 1) * 512], ps[:], bqk_sb[:, ct:ct + 1]
                )

            for st in range(4):
                qk_unit(0, st)
            head_singles = [(0, 0), (0, 1)]
            for st, j in head_singles:
                qk_unit_single(st, j)
            # E2: V tiles 0..3 (needed by the first PV steps)
            for t16 in range(4):
                v_unit(t16)

            # E3: groups; each group's carry (last 3 PV drains + normalize)
            # runs as the NEXT group's first fillers, so neither the PE nor
            # the Scalar engine idles at group boundaries.
            # qk-p1 singles: 2 in the head (above), 2 in (0,0), 4 in (0,1);
            # V 4..13 fill (0,0) (deadline: own PV); V 14,15 -> (1,0).
            def mkv(a):
                return lambda: (v_unit(a), v_unit(a + 1))

            def mkq(st, j):
                return lambda: qk_unit_single(st, j)

            qfill = deque(
                mkq(st, j)
                for st in range(4)
                for j in range(2)
                if (st, j) not in head_singles
            )
            fill0 = deque(mkv(a) for a in range(4, 14, 2))
            fill0.append(qfill.popleft())
            fill0.append(qfill.popleft())

            G = [(0, 0), (0, 1), (1, 0), (1, 1), (0, 2), (1, 2), (0, 3), (1, 3)]
            carry = []
            fins = {}
            for i, (pair, k) in enumerate(G):
                filler = deque(carry)
                if i == 0:
                    filler.extend(fill0)
                if i == 1:
                    filler.extend(qfill)
                if i == 2:
                    filler.append(mkv(14))
                st_g, carry = attn_group(pair, k, filler)
                fins[(pair, k)] = st_g
                # staging + collectives as soon as each phase's finishes exist
                if (pair, k) == (1, 1):
                    stage(0, a2a_in1, 0)
                if (pair, k) == (0, 2):
                    stage(1, a2a_in1, 1)
                    nc.gpsimd.collective_compute(
                        "AllToAll", mybir.AluOpType.bypass,
                        ins=[a2a_in1[:].opt()], outs=[a2a_out1[:].opt()],
                        replica_groups=groups,
                    )
                    pjT_unit(0, a2a_out1, 0, nc.sync)
                    pjT_unit(1, a2a_out1, 1, nc.sync)
                if (pair, k) == (0, 3):
                    stage(2, a2a_in2, 0)
            # gate derives from fin(0,3)'s aT output (emitted inside (1,3)):
            # ready before the last group ends, so proj{0,1} can follow the
            # last PV drains with no Vector dependency in between.  The gate
            # still pins proj behind the attention stream in the PE queue,
            # so a slow A2A#1 cannot head-block anything earlier.
            gate_sb = wpool.tile([1, P], BF16)
            nc.vector.tensor_scalar(
                out=gate_sb[:], in0=aT_sb[0:1, 0, 384:512],
                scalar1=0.0, scalar2=1.0,
                op0=mybir.AluOpType.mult, op1=mybir.AluOpType.add,
            )
            proj_unit(0, 0, gate_sb)  # no exp dependency: runs immediately
            for fn in carry:  # drains + finish of (1,3)
                fn()
            stage(3, a2a_in2, 1)
            nc.gpsimd.collective_compute(
                "AllToAll", mybir.AluOpType.bypass,
                ins=[a2a_in2[:].opt()], outs=[a2a_out2[:].opt()],
                replica_groups=groups,
            )
            # pjT for A2A#2 slots on the Scalar queue: the sync queue carries
            # the out DMAs, which must not wait behind collective #2
            pjT_unit(2, a2a_out2, 0, nc.scalar)
            pjT_unit(3, a2a_out2, 1, nc.scalar)
            proj_unit(0, 1, gate_sb)
            out_unit(0, 0)
            out_unit(0, 1)
            for slot in (1, 2, 3):
                for dc in range(2):
                    proj_unit(slot, dc, gate_sb)
                    out_unit(slot, dc)

    nc.compile()
    return nc


def make_in_maps(x, w_qkv, b_qkv, w_proj, b_proj):
    import ml_dtypes

    bf16 = ml_dtypes.bfloat16
    x = np.asarray(x, dtype=np.float32)
    w_qkv = np.asarray(w_qkv, dtype=np.float32)
    b_qkv = np.asarray(b_qkv, dtype=np.float32)
    w_proj_bf = np.ascontiguousarray(np.asarray(w_proj, dtype=np.float32).astype(bf16))
    b_proj_bf = np.ascontiguousarray(
        np.asarray(b_proj, dtype=np.float32).astype(bf16).reshape(1, -1)
    )

    # maskT[q_local, kv_local] = 0 if q >= kv else NEG (stationary operand of
    # the mask matmul: out[kv, q] += maskT[q, kv] via identity moving data)
    mT = np.where(np.arange(P)[:, None] >= np.arange(P)[None, :], 0.0, NEG)
    ident = np.eye(P)
    mi = np.ascontiguousarray(
        np.concatenate([mT, ident], axis=1).astype(bf16)
    )

    in_maps = []
    for c in range(N_CORES):
        b, g = divmod(c, 4)
        hs = slice(g * HPC * HD, (g + 1) * HPC * HD)
        xT = np.ascontiguousarray(x[b].T.astype(bf16))           # [D, S]
        w_q = w_qkv[:, 0:D][:, hs]
        w_k = w_qkv[:, D:2 * D][:, hs]
        # columns: [q_p0 | k_p0 | q_p1 | k_p1]
        w_qk = np.ascontiguousarray(np.concatenate(
            [w_q[:, 0:128], w_k[:, 0:128], w_q[:, 128:256], w_k[:, 128:256]], axis=1
        ).astype(bf16))
        w_v = np.ascontiguousarray(w_qkv[:, 2 * D:3 * D][:, hs].astype(bf16))
        bq = b_qkv[0:D][hs]
        bk = b_qkv[D:2 * D][hs]
        bqk = np.stack([bq[0:128], bk[0:128], bq[128:256], bk[128:256]], axis=1)  # [128, 4]
        cst = np.ascontiguousarray(bqk.astype(np.float32))
        bv = np.ascontiguousarray(b_qkv[2 * D:3 * D][hs].reshape(1, -1).astype(bf16))
        in_maps.append(
            {
                "xT": xT,
                "w_qk": w_qk,
                "w_v": w_v,
                "consts": cst,
                "mi": mi,
                "b_v": bv,
                "w_proj": w_proj_bf,
                "b_proj": b_proj_bf,
            }
        )
    return in_maps


_NC_CACHE = None


def _install_ntff_shim():
    """Provide the antenv.axon_hooks module bass_utils wants for trace=True."""
    import sys as _sys
    import types

    if "antenv.axon_hooks" in _sys.modules:
        return
    try:
        from trn_agent_boot.trn_boot import _ntff_profile_via_ctypes

        hook = _ntff_profile_via_ctypes("/opt/axon/libaxon_pjrt.so")
    except Exception:
        hook = None
    mod = types.ModuleType("antenv.axon_hooks")
    mod._hook = hook
    mod.get_axon_ntff_profile_hook = lambda: mod._hook
    mod.set_axon_ntff_profile_hook = lambda h: setattr(mod, "_hook", h)
    _sys.modules["antenv.axon_hooks"] = mod


def kernel(x, w_qkv, b_qkv, w_proj, b_proj):
    global _NC_CACHE
    if _NC_CACHE is None:
        _NC_CACHE = build()
    nc = _NC_CACHE
    in_maps = make_in_maps(x, w_qkv, b_qkv, w_proj, b_proj)
    trace = bool(int(os.environ.get("KERNEL_TRACE", "0")))
    if trace:
        _install_ntff_shim()
    res = run_bass_kernel_spmd(
        nc,
        in_maps,
        core_ids=list(range(N_CORES)),
        trace=trace,
    )
    out = np.empty((B, S, D), dtype=np.float32)
    for c in range(N_CORES):
        oc = res.results[c]["out"]  # [4, 128, 1024] bf16
        j, h = divmod(c, 2)
        for k in range(4):
            r0 = 512 * j + 128 * k + 64 * h
            out[0, r0:r0 + 64, :] = oc[k, 0:64, :].astype(np.float32)
            out[1, r0:r0 + 64, :] = oc[k, 64:128, :].astype(np.float32)
    kernel.last_results = res
    return out


# revision 39
# speedup vs baseline: 1.1257x; 1.0670x over previous
"""Distributed causal multi-head attention for 8 Trainium2 NeuronCores.

Problem: B=2, S=2048, D=1024, H=16 heads (hd=64), fp32.
    qkv = x @ w_qkv + b_qkv ; causal softmax attention ; out = attn @ w_proj + b_proj

Distribution: core c -> (batch b = c//4, head group g = c%4 -> heads [4g, 4g+4)).
Transposed dataflow (channels on partitions, sequence on free axis); x arrives
host-transposed.

Restructured from the v1 phase-serial kernel (~245-257us) to ~215-225us.
Key findings from trace analysis, baked into this version:
  - k-phase output slots: each k-phase's attention output ships in 64-row
    "slots": receiver core c gets, per phase k, rows
    [512*(c//2) + 128*k + 64*(c%2), +64) of both batches.  A2A#1 carries
    phases {0,1} (doorbell ~mid-attention), A2A#2 carries {2,3} (doorbell at
    attention end).  Collectives cost ~15-25us of data movement PLUS ~27us
    of queue-release lag on the serializing GpSimd queue, and consumers see
    completion ~5us after the data phase ends; two A2As with all projection
    work at the end is the best overlap under those constants (3-4 smaller
    A2As cascade on the queue lag; 4-core-group A2As are unsupported - mesh
    needs >4 ranks).
  - A tiny warm-up AllGather issues at t~0: the FIRST collective pays
    ~55-60us extra firmware warm-up; this absorbs it off the critical path.
    Nothing else may use the GpSimd queue (it blocks behind collectives),
    so the V' ones-memset runs on Vector.
  - Projection packs batch0 rows on partitions 0:64 and batch1 on 64:128 so
    proj matmuls keep full 128-partition output despite 64-row slots.
  - The causal mask is applied INSIDE the score PSUM accumulation as an
    extra matmul (mask^T stationary x identity moving) on diagonal tiles:
    a Vector-engine mask add would queue behind the normalize chains and
    delay exp by up to ~3us per group (the Vector queue is in-order).
  - Each group's last 3 PV chunks and its normalize chain are carried into
    the NEXT group as its first 4 fillers, so neither the PE nor Scalar
    idles at group boundaries (boundary stalls also reset the PE p-state:
    it runs at 1.2GHz until ~3us of continuous execution, 2.4GHz after).
  - The softmax 1/sum broadcast matmul is bf16 (fp32 matmul is 4 cyc/row:
    853ns x16 = 13.6us of PE busy; bf16 is 213ns).
  - qk pair-1 and V-tile projections run as fillers inside the first two
    groups (pair-swapped order (0,0),(0,1),(1,0),(1,1),...), shrinking the
    serial pre-attention head.
  - pjT fetches for A2A#2 slots issue on the Scalar queue so they cannot
    head-block staging/out DMAs on the sync queue; pjT fetches for A2A#1
    prefetch on sync right after the collective.
  - Priority-ordered DMA with a small first chunk (wqk d0 + x slab0 d0);
    output ships as bf16 (host casts back to f32).
"""

import os
import sys
from collections import deque

sys.path.insert(0, "/opt/trn_rl_repo")

import numpy as np

import concourse.bass as bass
import concourse.tile as tile
from concourse import bacc, mybir
from concourse.bass_utils import run_bass_kernel_spmd

B, S, D = 2, 2048, 1024
H = 16
HD = 64
P = 128
N_CORES = 8
HPC = 4           # heads per core
DCH = D // P      # 8 contraction chunks
NQT = S // P      # 16 q tiles of 128
SCALE = 1.0 / 8.0  # 1/sqrt(hd)
NEG = -1.0e9

F32 = mybir.dt.float32
BF16 = mybir.dt.bfloat16


def attn_chunks(k):
    """Pack the kv-tile iterations of q-group k into <=512-col score chunks.

    First-fit: a late short tile (N=128) fills the slack of an earlier
    384-col chunk, so each chunk (= one exp call) is as full as possible."""
    T = 13 + k
    chunks = []  # [entries, used]
    for t in range(T):
        s0 = max(0, (t - k + 3) // 4)
        N = (4 - s0) * P
        for ch in chunks:
            if ch[1] + N <= 512:
                ch[0].append((t, ch[1], N, s0))
                ch[1] += N
                break
        else:
            chunks.append([[(t, 0, N, s0)], N])
    out = [c[0] for c in chunks]
    # split the final 512-col chunk into two halves: its exp call then
    # finishes sooner after its scores, so the next group's second score
    # chunk never stalls on the 2-deep PSUM score ring
    last = out[-1]
    if len(last) == 2:
        out[-1] = [last[0]]
        out.append([(last[1][0], 0, last[1][2], last[1][3])])
    return out


def build():
    nc = bacc.Bacc(num_devices=N_CORES)

    xT = nc.declare_dram_parameter("xT", [D, S], BF16, isOutput=False)
    # columns reordered host-side: [q_p0 | k_p0 | q_p1 | k_p1] (128 each)
    w_qk = nc.declare_dram_parameter("w_qk", [D, 2 * HPC * HD], BF16, isOutput=False)
    w_v = nc.declare_dram_parameter("w_v", [D, HPC * HD], BF16, isOutput=False)
    consts = nc.declare_dram_parameter("consts", [P, 4], F32, isOutput=False)
    # mi[:, 0:128] = mask^T (0 where q>=kv else NEG), mi[:, 128:256] = identity
    mi = nc.declare_dram_parameter("mi", [P, 2 * P], BF16, isOutput=False)
    b_v = nc.declare_dram_parameter("b_v", [1, HPC * HD], BF16, isOutput=False)
    w_proj = nc.declare_dram_parameter("w_proj", [D, D], BF16, isOutput=False)
    b_proj = nc.declare_dram_parameter("b_proj", [1, D], BF16, isOutput=False)
    # slot k: rows [512*(c//2) + 128*k + 64*(c%2), +64); partitions 0:64 = b0,
    # 64:128 = b1
    out_ext = nc.declare_dram_parameter("out", [4, P, D], BF16, isOutput=True)

    groups = [list(range(N_CORES))]

    with tile.TileContext(nc) as tc:
        with (
            tc.tile_pool(name="weights", bufs=1) as wpool,
            tc.tile_pool(name="xslab", bufs=4) as xpool,
            tc.tile_pool(name="qkT", bufs=1) as qkpool,
            tc.tile_pool(name="big", bufs=1) as bigpool,
            tc.tile_pool(name="prob", bufs=7) as ppool,
            tc.tile_pool(name="small", bufs=4) as spool,
            tc.tile_pool(name="pj", bufs=4) as pjpool,
            tc.tile_pool(name="dram", bufs=1, space="DRAM") as dpool,
            tc.tile_pool(name="psS", bufs=2, space="PSUM") as psS,   # scores 2 banks/slot
            tc.tile_pool(name="psV", bufs=2, space="PSUM") as psV,   # pv accumulators
            tc.tile_pool(name="psQ", bufs=2, space="PSUM") as psQ,   # qkv/proj groups
        ):
            # a2a layout: [dest core, bslot*256 + pair*128 + hd_part, 64 rows]
            # A2A#1 carries phases {0,1} (doorbell ~mid-attention; its ~27us
            # queue-release lag mostly clears before A2A#2's data exists);
            # A2A#2 carries phases {2,3} at attention end.
            a2a_in1 = dpool.tile([N_CORES, 512, 64], BF16, tag="a2a_in1")
            a2a_out1 = dpool.tile([N_CORES, 512, 64], BF16, tag="a2a_out1")
            a2a_in2 = dpool.tile([N_CORES, 512, 64], BF16, tag="a2a_in2")
            a2a_out2 = dpool.tile([N_CORES, 512, 64], BF16, tag="a2a_out2")
            dummy_in = dpool.tile([1, 16], BF16, tag="dummy_in")
            dummy_out = dpool.tile([N_CORES, 16], BF16, tag="dummy_out")

            # tiny warm-up collective issued immediately: the collectives
            # firmware has a large first-collective warm-up cost (observed:
            # the first real A2A would not move data before ~145us regardless
            # of when its inputs were staged); this absorbs it off the
            # critical path.
            nc.sync.dma_start(out=dummy_in[:], in_=b_v[:, 0:16])
            nc.gpsimd.collective_compute(
                "AllGather", mybir.AluOpType.bypass,
                ins=[dummy_in[:].opt()], outs=[dummy_out[:].opt()],
                replica_groups=groups,
            )

            # ---- DMA priority order ----
            wqk_sb = wpool.tile([P, DCH, 2 * HPC * HD], BF16)
            xsl_list = [
                xpool.tile([P, DCH, 512], BF16, tag="xslab", name=f"xsl{st}")
                for st in range(4)
            ]
            # smallest possible first chunks so the first matmul starts early
            nc.sync.dma_start(
                out=wqk_sb[:, 0:1, 0:256],
                in_=w_qk[:, 0:256].rearrange("(o p) c -> p o c", p=P)[:, 0:1, :],
            )
            nc.sync.dma_start(
                out=xsl_list[0][:, 0:1, :],
                in_=xT[:, :].rearrange("(o p) s -> p o s", p=P)[:, 0:1, 0:512],
            )
            consts_sb = wpool.tile([P, 4], F32)
            nc.sync.dma_start(out=consts_sb[:], in_=consts[:, :])
            nc.sync.dma_start(
                out=wqk_sb[:, 1:4, 0:256],
                in_=w_qk[:, 0:256].rearrange("(o p) c -> p o c", p=P)[:, 1:4, :],
            )
            nc.sync.dma_start(
                out=xsl_list[0][:, 1:4, :],
                in_=xT[:, :].rearrange("(o p) s -> p o s", p=P)[:, 1:4, 0:512],
            )
            nc.sync.dma_start(
                out=wqk_sb[:, 4:8, 0:256],
                in_=w_qk[:, 0:256].rearrange("(o p) c -> p o c", p=P)[:, 4:8, :],
            )
            nc.sync.dma_start(
                out=xsl_list[0][:, 4:8, :],
                in_=xT[:, :].rearrange("(o p) s -> p o s", p=P)[:, 4:8, 0:512],
            )
            mi_sb = wpool.tile([P, 2 * P], BF16)
            nc.sync.dma_start(out=mi_sb[:], in_=mi[:, :])
            nc.sync.dma_start(
                out=wqk_sb[:, :, 256:512],
                in_=w_qk[:, 256:512].rearrange("(o p) c -> p o c", p=P),
            )
            for st in range(1, 4):
                for dh in range(2):
                    dsl = slice(dh * 4, dh * 4 + 4)
                    nc.sync.dma_start(
                        out=xsl_list[st][:, dsl, :],
                        in_=xT[:, :].rearrange("(o p) s -> p o s", p=P)[:, dsl, st * 512:(st + 1) * 512],
                    )
            wv_sb = wpool.tile([P, DCH, HPC * HD], BF16)
            nc.sync.dma_start(out=wv_sb[:], in_=w_v[:, :].rearrange("(o p) c -> p o c", p=P))
            wproj_sb = wpool.tile([P, DCH, D], BF16)
            nc.sync.dma_start(out=wproj_sb[:], in_=w_proj[:, :].rearrange("(o p) c -> p o c", p=P))
            bproj_sb = wpool.tile([1, D], BF16)
            nc.sync.dma_start(out=bproj_sb[:], in_=b_proj[:, :])

            bqk_sb = consts_sb[:, 0:4]
            maskT_sb = mi_sb[:, 0:P]
            ident_sb = mi_sb[:, P:2 * P]
            ones_sb = wpool.tile([1, P], BF16)
            nc.vector.memset(ones_sb[:], 1.0)
            warm_sb = wpool.tile([1, 512], BF16)
            nc.vector.memset(warm_sb[:], 1.0)
            # ~10 throwaway matmuls: the PE clock needs ~3us of continuous
            # execution to ramp 0.65->2.4GHz; these run while the first
            # weight DMAs land so the real qk units start at speed
            for _ in range(10):
                wps = psQ.tile([P, 512], F32, tag="mm", name="warm")
                nc.tensor.matmul(
                    wps[:], ones_sb[:, :], warm_sb[:, :], start=True, stop=True,
                )

            # qkT layout: [128, ct, 2048]; ct: 0=q_p0, 1=k_p0, 2=q_p1, 3=k_p1
            qkT_sb = qkpool.tile([P, 4, S], BF16)
            # V': [128 kv_inner, 16 kv_outer, 4*65]; col 65h+64 = 1.0 (softmax denom)
            # memset on Vector, NOT GpSimd: the GpSimd queue must stay free
            # for collectives (the warm-up AllGather may occupy it for a
            # long time)
            v1_sb = bigpool.tile([P, NQT, HPC * 65], BF16)
            nc.vector.memset(v1_sb[:], 1.0)
            # attn outT: [128 (2 heads x 64), pair, 2048] bf16; q tile 4j+k at
            # col j*512 + k*128
            aT_sb = bigpool.tile([P, 2, S], BF16)
            # proj out: partitions 0:64 batch0 rows, 64:128 batch1 rows
            out_sb = bigpool.tile([P, 4, D], BF16)

            # ---- unit emitters ----
            def qk_unit(pair, st):
                """q,k projection for one pair, one s-slab (two 128-col tiles)."""
                xsl = xsl_list[st]
                for j in range(2):
                    ct = 2 * pair + j
                    ps = psQ.tile([P, 512], F32, tag="mm")
                    for d in range(DCH):
                        nc.tensor.matmul(
                            ps[:],
                            wqk_sb[:, d, ct * P:(ct + 1) * P],
                            xsl[:, d, :],
                            start=(d == 0),
                            stop=(d == DCH - 1),
                        )
                    nc.vector.tensor_scalar_add(
                        qkT_sb[:, ct, st * 512:(st + 1) * 512], ps[:], bqk_sb[:, ct:ct + 1]
                    )

            def v_unit(t16):
                """V projection for one 128-row s-tile (all 4 heads)."""
                st, sq = divmod(t16, 4)
                xsl = xsl_list[st]
                ps_full = psQ.tile([P, 512], F32, tag="mm", name="vacc")
                ps = ps_full[:, :HPC * HD]
                # no V-bias matmul: softmax weights sum to 1, so the V bias
                # is a constant row of attn_out and bv @ w_proj is folded
                # into b_proj host-side
                for d in range(DCH):
                    nc.tensor.matmul(
                        ps[:],
                        xsl[:, d, sq * P:(sq + 1) * P],
                        wv_sb[:, d, :],
                        start=(d == 0),
                        stop=(d == DCH - 1),
                    )
                for h in range(HPC):
                    nc.vector.tensor_copy(
                        out=v1_sb[:, t16, h * 65:h * 65 + HD],
                        in_=ps[:, h * HD:(h + 1) * HD],
                    )

            # ---- attention group: scores/exp/PV with chunk-packed exp and
            # lag-2 PV.  The LAST 2 PV chunks and the normalize chain are NOT
            # emitted here: they return as carry closures that the next group
            # runs as its first fillers.  This removes the group-boundary
            # serialization (v2: PE waited ~2-4us at every boundary for
            # exp(last)->PV(last) before starting the next group's scores).
            def attn_group(pair, k, filler):
                chunks = attn_chunks(k)
                first_t = chunks[0][0][0]
                last_t = chunks[-1][-1][0]
                qvA = qkT_sb[0:HD, 2 * pair, :].rearrange("p (i g) -> p i g", g=512)
                qvB = qkT_sb[HD:P, 2 * pair, :].rearrange("p (i g) -> p i g", g=512)
                kv_ct = 2 * pair + 1
                pvA = psV.tile([P, 512], F32, tag="pv")
                pvB = psV.tile([P, 512], F32, tag="pv")

                def emit_pv(pr, ch):
                    for (t, off, N, s0) in ch:
                        for hh, pv in ((0, pvA), (1, pvB)):
                            h = 2 * pair + hh
                            nc.tensor.matmul(
                                pv[0:65, s0 * P:512],
                                v1_sb[:, t, h * 65:(h + 1) * 65],
                                pr[:, hh, off:off + N],
                                start=(t == first_t), stop=(t == last_t),
                            )

                pending = deque()
                for ci, ch in enumerate(chunks):
                    used = ch[-1][1] + ch[-1][2]
                    sc_full = psS.tile([P, 2, 512], F32, tag="sc")
                    for (t, off, N, s0) in ch:
                        sc = sc_full[:, :, off:off + N]
                        # diagonal tile: fold the causal mask into the PSUM
                        # accumulation (mask^T stationary x identity moving) so
                        # the score->exp path never touches the Vector queue
                        diag = t >= k and (t - k) % 4 == 0
                        nc.tensor.matmul(
                            sc[:, 0, :],
                            qkT_sb[0:HD, kv_ct, t * P:(t + 1) * P],
                            qvA[:, s0:4, k * P:(k + 1) * P],
                            start=True, stop=not diag, tile_position=(0, 0),
                        )
                        nc.tensor.matmul(
                            sc[:, 1, :],
                            qkT_sb[HD:P, kv_ct, t * P:(t + 1) * P],
                            qvB[:, s0:4, k * P:(k + 1) * P],
                            start=True, stop=not diag, tile_position=(64, 0),
                        )
                        if diag:
                            for hh in range(2):
                                nc.tensor.matmul(
                                    sc_full[:, hh, off:off + P],
                                    maskT_sb[:, :], ident_sb[:, :],
                                    start=False, stop=True,
                                )
                    pr = ppool.tile([P, 2, 512], BF16, tag="prob")
                    nc.scalar.activation(
                        pr[:, :, 0:used], sc_full[:, :, 0:used],
                        mybir.ActivationFunctionType.Exp, scale=SCALE,
                    )
                    if filler:
                        filler.popleft()()
                    if len(pending) >= 3:
                        emit_pv(*pending.popleft())
                    pending.append((pr, ch))
                while filler:
                    filler.popleft()()
                st = (pair, k, pvA, pvB)

                def mk_drain():
                    item = pending.popleft()
                    return lambda: emit_pv(*item)

                # carry: [drain PV x3, finish_a, finish_b]; popped at chunks
                # 0..4 of the next group, so the last drain trails exp(c_last)
                # by ~2 chunks of PE work and the bc matmul (finish_b) trails
                # the Vector reciprocal chain (finish_a) by one chunk
                cell = []
                carry = [
                    mk_drain(), mk_drain(), mk_drain(),
                    lambda: finish_a(st, cell),
                    lambda: finish_b(st, cell),
                ]
                return st, carry

            def finish_a(st, cell):
                # Vector-only half of the normalize: runs one filler slot
                # before finish_b so the bc matmul never waits on this chain
                pair, k, pvA, pvB = st
                sums = spool.tile([1, 2, 512], F32, tag="sums")
                nc.vector.tensor_copy(out=sums[:, 0, :], in_=pvA[64:65, :])
                nc.vector.tensor_copy(out=sums[:, 1, :], in_=pvB[64:65, :])
                # bf16 copies release the pv PSUM slots promptly
                pvc = spool.tile([HD, 2, 512], BF16, tag="pvc")
                nc.vector.tensor_copy(out=pvc[:, 0, :], in_=pvA[0:HD, :])
                nc.vector.tensor_copy(out=pvc[:, 1, :], in_=pvB[0:HD, :])
                rec = spool.tile([1, 2, 512], F32, tag="rec")
                nc.vector.reciprocal_approx_fast(rec[:], sums[:])
                recb = spool.tile([1, 2, 512], BF16, tag="recb")
                nc.vector.tensor_copy(out=recb[:], in_=rec[:])
                cell.append((pvc, recb))

            def finish_b(st, cell):
                pair, k, pvA, pvB = st
                pvc, recb = cell[0]
                for hh in range(2):
                    base = hh * HD
                    # broadcast 1/den across the 64 hd partitions with a K=1
                    # bf16 matmul (keeps GpSimd out of the normalize path)
                    bc = psQ.tile([P, 512], F32, tag="mm", name="bc")
                    nc.tensor.matmul(
                        bc[0:HD, :], ones_sb[:, 0:HD], recb[:, hh, :],
                        start=True, stop=True,
                    )
                    nc.vector.tensor_tensor(
                        out=aT_sb[base:base + HD, pair, :]
                        .rearrange("p (j q) -> p j q", q=4 * P)[:, :, k * P:(k + 1) * P],
                        in0=pvc[:, hh, :].rearrange("p (j f) -> p j f", f=P),
                        in1=bc[0:HD, :].rearrange("p (j f) -> p j f", f=P),
                        op=mybir.AluOpType.mult,
                    )

            def stage(k, buf, slot):
                # phase k: q tile 4j+k, 64-row half h -> dest core 2j+h
                for t0 in range(2):
                    for h in range(2):
                        nc.sync.dma_start(
                            out=buf[:, slot * 256 + t0 * P:slot * 256 + (t0 + 1) * P, :]
                            .rearrange("(j h) pp f -> j h pp f", h=2)[:, h]
                            .rearrange("j pp f -> pp j f"),
                            in_=aT_sb[:, t0, :]
                            .rearrange("pp (j q) -> pp j q", q=4 * P)
                            [:, :, k * P + 64 * h:k * P + 64 * h + 64],
                        )

            pjT_tiles = {}

            def pjT_unit(slot, buf, bslot, eng):
                pjT = pjpool.tile([P, DCH, P], BF16, tag="pjT", name=f"pjT{slot}")
                for t in range(2):
                    for b in range(2):
                        eng.dma_start(
                            out=pjT[:, :, b * 64:(b + 1) * 64]
                            .rearrange("pp (g t) f -> pp g t f", t=2)[:, :, t]
                            .rearrange("pp g f -> pp g f"),
                            in_=buf[4 * b:4 * b + 4,
                                    bslot * 256 + t * P:bslot * 256 + (t + 1) * P, :]
                            .rearrange("g pp f -> pp g f"),
                        )
                pjT_tiles[slot] = pjT

            def proj_unit(slot, dc, gate):
                pjT = pjT_tiles[slot]
                ps = psQ.tile([P, 512], F32, tag="mm", name="pacc")
                nc.tensor.matmul(
                    ps[:], gate[:, :],
                    bproj_sb[:, dc * 512:(dc + 1) * 512],
                    start=True, stop=False,
                )
                for ch in range(DCH):
                    nc.tensor.matmul(
                        ps[:],
                        pjT[:, ch, :],
                        wproj_sb[:, ch, dc * 512:(dc + 1) * 512],
                        start=False,
                        stop=(ch == DCH - 1),
                    )
                nc.vector.tensor_copy(out=out_sb[:, slot, dc * 512:(dc + 1) * 512], in_=ps[:])

            def out_unit(slot, dc):
                nc.sync.dma_start(
                    out=out_ext[slot, :, dc * 512:(dc + 1) * 512],
                    in_=out_sb[:, slot, dc * 512:(dc + 1) * 512],
                )

            # ---- emission schedule ----
            # E1: qk pair 0 only; pair 1 runs as fillers inside (0,0)/(0,1)
            # (pair-swapped first half), shrinking the serial head by ~14us.
            def qk_unit_single(st, j):
                xsl = xsl_list[st]
                ct = 2 + j
                ps = psQ.tile([P, 512], F32, tag="mm")
                for d in range(DCH):
                    nc.tensor.matmul(
                        ps[:],
                        wqk_sb[:, d, ct * P:(ct + 1) * P],
                        xsl[:, d, :],
                        start=(d == 0),
                        stop=(d == DCH - 1),
                    )
                nc.vector.tensor_scalar_add(
                    qkT_sb[:, ct, st * 512:(st + 1) * 512], ps[:], bqk_sb[:, ct:ct + 1]
                )

            for st in range(4):
                qk_unit(0, st)
            head_singles = [(0, 0)]
            for st, j in head_singles:
                qk_unit_single(st, j)
            # E2: V tiles 0..3 (needed by the first PV steps)
            for t16 in range(4):
                v_unit(t16)

            # E3: groups; each group's carry (last 3 PV drains + normalize)
            # runs as the NEXT group's first fillers, so neither the PE nor
            # the Scalar engine idles at group boundaries.
            # qk-p1 singles: 2 in the head (above), 2 in (0,0), 4 in (0,1);
            # V 4..13 fill (0,0) (deadline: own PV); V 14,15 -> (1,0).
            def mkv(a):
                return lambda: (v_unit(a), v_unit(a + 1))

            def mkq(st, j):
                return lambda: qk_unit_single(st, j)

            qfill = deque(
                mkq(st, j)
                for st in range(4)
                for j in range(2)
                if (st, j) not in head_singles
            )
            fill0 = deque(mkv(a) for a in range(4, 14, 2))
            fill0.append(qfill.popleft())
            fill0.append(qfill.popleft())
            fill0.append(qfill.popleft())

            G = [(0, 0), (0, 1), (1, 0), (1, 1), (0, 2), (1, 2), (0, 3), (1, 3)]
            carry = []
            fins = {}
            for i, (pair, k) in enumerate(G):
                filler = deque(carry)
                if i == 0:
                    filler.extend(fill0)
                if i == 1:
                    filler.extend(qfill)
                if i == 2:
                    filler.append(mkv(14))
                st_g, carry = attn_group(pair, k, filler)
                fins[(pair, k)] = st_g
                # staging + collectives as soon as each phase's finishes exist
                if (pair, k) == (1, 1):
                    stage(0, a2a_in1, 0)
                if (pair, k) == (0, 2):
                    stage(1, a2a_in1, 1)
                    nc.gpsimd.collective_compute(
                        "AllToAll", mybir.AluOpType.bypass,
                        ins=[a2a_in1[:].opt()], outs=[a2a_out1[:].opt()],
                        replica_groups=groups,
                    )
                    pjT_unit(0, a2a_out1, 0, nc.sync)
                    pjT_unit(1, a2a_out1, 1, nc.sync)
                if (pair, k) == (0, 3):
                    stage(2, a2a_in2, 0)
            # gate derives from fin(0,3)'s aT output (emitted inside (1,3)):
            # ready before the last group ends, so proj{0,1} can follow the
            # last PV drains with no Vector dependency in between.  The gate
            # still pins proj behind the attention stream in the PE queue,
            # so a slow A2A#1 cannot head-block anything earlier.
            gate_sb = wpool.tile([1, P], BF16)
            nc.vector.tensor_scalar(
                out=gate_sb[:], in0=aT_sb[0:1, 0, 384:512],
                scalar1=0.0, scalar2=1.0,
                op0=mybir.AluOpType.mult, op1=mybir.AluOpType.add,
            )
            proj_unit(0, 0, gate_sb)  # no exp dependency: runs immediately
            for fn in carry:  # drains + finish of (1,3)
                fn()
            stage(3, a2a_in2, 1)
            nc.gpsimd.collective_compute(
                "AllToAll", mybir.AluOpType.bypass,
                ins=[a2a_in2[:].opt()], outs=[a2a_out2[:].opt()],
                replica_groups=groups,
            )
            # pjT for A2A#2 slots on the Scalar queue: the sync queue carries
            # the out DMAs, which must not wait behind collective #2
            pjT_unit(2, a2a_out2, 0, nc.scalar)
            pjT_unit(3, a2a_out2, 1, nc.scalar)
            proj_unit(0, 1, gate_sb)
            out_unit(0, 0)
            out_unit(0, 1)
            for slot in (1, 2, 3):
                for dc in range(2):
                    proj_unit(slot, dc, gate_sb)
                    out_unit(slot, dc)

    nc.compile()
    return nc


def make_in_maps(x, w_qkv, b_qkv, w_proj, b_proj):
    import ml_dtypes

    bf16 = ml_dtypes.bfloat16
    x = np.asarray(x, dtype=np.float32)
    w_qkv = np.asarray(w_qkv, dtype=np.float32)
    b_qkv = np.asarray(b_qkv, dtype=np.float32)
    w_proj_bf = np.ascontiguousarray(np.asarray(w_proj, dtype=np.float32).astype(bf16))
    # softmax weights sum to 1, so the V bias contributes bv @ w_proj to
    # every output row; fold it into the projection bias
    bv_full = np.asarray(b_qkv, dtype=np.float32)[2 * D:3 * D]
    b_proj_eff = np.asarray(b_proj, dtype=np.float32) + bv_full @ np.asarray(
        w_proj, dtype=np.float32
    )
    b_proj_bf = np.ascontiguousarray(b_proj_eff.astype(bf16).reshape(1, -1))

    # maskT[q_local, kv_local] = 0 if q >= kv else NEG (stationary operand of
    # the mask matmul: out[kv, q] += maskT[q, kv] via identity moving data)
    mT = np.where(np.arange(P)[:, None] >= np.arange(P)[None, :], 0.0, NEG)
    ident = np.eye(P)
    mi = np.ascontiguousarray(
        np.concatenate([mT, ident], axis=1).astype(bf16)
    )

    in_maps = []
    for c in range(N_CORES):
        b, g = divmod(c, 4)
        hs = slice(g * HPC * HD, (g + 1) * HPC * HD)
        xT = np.ascontiguousarray(x[b].T.astype(bf16))           # [D, S]
        w_q = w_qkv[:, 0:D][:, hs]
        w_k = w_qkv[:, D:2 * D][:, hs]
        # columns: [q_p0 | k_p0 | q_p1 | k_p1]
        w_qk = np.ascontiguousarray(np.concatenate(
            [w_q[:, 0:128], w_k[:, 0:128], w_q[:, 128:256], w_k[:, 128:256]], axis=1
        ).astype(bf16))
        w_v = np.ascontiguousarray(w_qkv[:, 2 * D:3 * D][:, hs].astype(bf16))
        bq = b_qkv[0:D][hs]
        bk = b_qkv[D:2 * D][hs]
        bqk = np.stack([bq[0:128], bk[0:128], bq[128:256], bk[128:256]], axis=1)  # [128, 4]
        cst = np.ascontiguousarray(bqk.astype(np.float32))
        bv = np.ascontiguousarray(b_qkv[2 * D:3 * D][hs].reshape(1, -1).astype(bf16))
        in_maps.append(
            {
                "xT": xT,
                "w_qk": w_qk,
                "w_v": w_v,
                "consts": cst,
                "mi": mi,
                "b_v": bv,
                "w_proj": w_proj_bf,
                "b_proj": b_proj_bf,
            }
        )
    return in_maps


_NC_CACHE = None


def _install_ntff_shim():
    """Provide the antenv.axon_hooks module bass_utils wants for trace=True."""
    import sys as _sys
    import types

    if "antenv.axon_hooks" in _sys.modules:
        return
    try:
        from trn_agent_boot.trn_boot import _ntff_profile_via_ctypes

        hook = _ntff_profile_via_ctypes("/opt/axon/libaxon_pjrt.so")
    except Exception:
        hook = None
    mod = types.ModuleType("antenv.axon_hooks")
    mod._hook = hook
    mod.get_axon_ntff_profile_hook = lambda: mod._hook
    mod.set_axon_ntff_profile_hook = lambda h: setattr(mod, "_hook", h)
    _sys.modules["antenv.axon_hooks"] = mod


def kernel(x, w_qkv, b_qkv, w_proj, b_proj):
    global _NC_CACHE
    if _NC_CACHE is None:
        _NC_CACHE = build()
    nc = _NC_CACHE
    in_maps = make_in_maps(x, w_qkv, b_qkv, w_proj, b_proj)
    trace = bool(int(os.environ.get("KERNEL_TRACE", "0")))
    if trace:
        _install_ntff_shim()
    res = run_bass_kernel_spmd(
        nc,
        in_maps,
        core_ids=list(range(N_CORES)),
        trace=trace,
    )
    out = np.empty((B, S, D), dtype=np.float32)
    for c in range(N_CORES):
        oc = res.results[c]["out"]  # [4, 128, 1024] bf16
        j, h = divmod(c, 2)
        for k in range(4):
            r0 = 512 * j + 128 * k + 64 * h
            out[0, r0:r0 + 64, :] = oc[k, 0:64, :].astype(np.float32)
            out[1, r0:r0 + 64, :] = oc[k, 64:128, :].astype(np.float32)
    kernel.last_results = res
    return out
